# revision 25
# baseline (speedup 1.0000x reference)
"""Trainium2 Bass kernel for the dual-stream transformer block
(nn_Block_87840671138274).

Sharding: 8 cores = 4 batches x 2 streams. Core i handles batch i//2,
stream i%2 (0=x, 1=y) and produces that stream's full output. Each core
redundantly computes the *other* stream's LN + K/V projections (~12%
extra FLOPs) so there are zero collectives.

Layout: everything transposed ([D, S] with D on SBUF partitions).
- Host pre-transposes inputs and pre-folds LN gamma/beta + softmax SCALE
  into the projection weights/biases (exact algebra, f32).
- LN stats (mean / mean-of-squares) via ones-matmul partition reductions.
- Attention computes s^T = k^T.T @ q^T per head (K=64), exp on ACT, and
  the softmax denominator comes from a ones-column appended to V
  (natural layout), accumulated in the same PSUM matmul as the context.
- No max-subtraction in softmax: scores are ~N(0, 0.31), |s| < ~2.
- bf16 matmuls with f32 PSUM accumulation; residual stream kept f32.
"""
import os
import numpy as np
import ml_dtypes

P = 128
S = 1024
D = 768
F = 3072
NH = 12
HD = 64
KT = D // P     # 6
JT = S // P     # 8
FT = F // P     # 24
EPS = 1e-6
SCALE = np.float32(1.0 / np.sqrt(HD))
BF = ml_dtypes.bfloat16

_PROGRAM = None


def _build_program():
    import concourse.bass as bass
    import concourse.bacc as bacc
    import concourse.tile as tile
    from concourse import mybir
    from contextlib import ExitStack

    f32 = mybir.dt.float32
    bf16 = mybir.dt.bfloat16
    Ax = mybir.AluOpType
    Act = mybir.ActivationFunctionType

    nc = bacc.Bacc("TRN2", target_bir_lowering=False, debug=False, num_devices=8)

    aT_d = nc.dram_tensor("aT", [D, S], f32, kind="ExternalInput").ap()
    oT_d = nc.dram_tensor("oT", [D, S], bf16, kind="ExternalInput").ap()
    w_d = {}
    for w in ("wq", "wk", "wv", "wk2", "wv2", "wo"):
        w_d[w] = nc.dram_tensor(w, [D, D], bf16, kind="ExternalInput").ap()
    w_d["w1"] = nc.dram_tensor("w1", [D, F], bf16, kind="ExternalInput").ap()
    w_d["w2"] = nc.dram_tensor("w2", [F, D], bf16, kind="ExternalInput").ap()
    bq_d = nc.dram_tensor("bq", [D, 1], f32, kind="ExternalInput").ap()
    bk_d = nc.dram_tensor("bk", [D, 1], f32, kind="ExternalInput").ap()
    bk2_d = nc.dram_tensor("bk2", [D, 1], f32, kind="ExternalInput").ap()
    bo_d = nc.dram_tensor("bo", [D, 1], f32, kind="ExternalInput").ap()
    b2_d = nc.dram_tensor("b2", [D, 1], f32, kind="ExternalInput").ap()
    b1_d = nc.dram_tensor("b1", [F, 1], f32, kind="ExternalInput").ap()
    bvrow_d = nc.dram_tensor("bvrow", [1, D], bf16, kind="ExternalInput").ap()
    bv2row_d = nc.dram_tensor("bv2row", [1, D], bf16, kind="ExternalInput").ap()
    wsc_d = nc.dram_tensor("wsc", [1, 2], f32, kind="ExternalInput").ap()
    out_d = nc.dram_tensor("outT", [D, S], f32, kind="ExternalOutput").ap()
    dbg = {}
    if os.environ.get("KERNEL_DEBUG"):
        for nm in ("dbgq", "dbgk", "dbgv", "dbgp", "dbgc"):
            dbg[nm] = nc.dram_tensor(nm, [P, S], bf16, kind="ExternalOutput").ap()

    with tile.TileContext(nc) as tc:
        with ExitStack() as ctx:
            perm = ctx.enter_context(tc.tile_pool(name="perm", bufs=1))
            res_pool = ctx.enter_context(tc.tile_pool(name="res", bufs=7))
            oT_pool = ctx.enter_context(tc.tile_pool(name="oTp", bufs=6))
            bb = ctx.enter_context(tc.tile_pool(name="bigbf", bufs=54))
            fw = ctx.enter_context(tc.tile_pool(name="f32w", bufs=7))
            wpool = ctx.enter_context(tc.tile_pool(name="wpool", bufs=12))

            def bbt(name, shape=(P, S)):
                return bb.tile(list(shape), bf16, name=name, tag="bb")

            def fwt(name, shape=(P, S)):
                return fw.tile(list(shape), f32, name=name, tag="fw")

            def act_recip(out_ap, in_ap):
                # Table-based reciprocal on the (idle) Scalar engine. The
                # nc.scalar.activation wrapper refuses Reciprocal for accuracy
                # reasons; here the inputs are well-conditioned positives
                # (softmax denominators ~1e3, LN std ~1) and table accuracy is
                # far inside this kernel's error budget -- and it takes ~1us
                # vs 6.5us for the exact DVE reciprocal, off the DVE critical
                # path.
                se = nc.scalar
                se.add_instruction(mybir.InstActivation(
                    name=nc.get_next_instruction_name(),
                    func=Act.Reciprocal,
                    ins=[se.lower_ap(in_ap),
                         mybir.ImmediateValue(dtype=f32, value=0.0),
                         mybir.ImmediateValue(dtype=f32, value=1.0),
                         mybir.ImmediateValue(dtype=f32, value=0.0)],
                    outs=[se.lower_ap(out_ap)]))

            ones_kk = perm.tile([P, P], bf16, name="ones_kk")
            nc.gpsimd.memset(ones_kk[:], 1.0)
            ones_row = perm.tile([1, P], bf16, name="ones_row")
            nc.gpsimd.memset(ones_row[:], 1.0)
            eps_t = perm.tile([P, 1], f32, name="eps_t")
            nc.gpsimd.memset(eps_t[:], float(EPS))

            def bias_tile(name, dram, nt):
                t = perm.tile([P, nt], f32, name=name)
                nc.sync.dma_start(t[:], dram.rearrange("(t p) o -> p (t o)", p=P))
                return t

            bq_t = bias_tile("bq_t", bq_d, KT)
            bk_t = bias_tile("bk_t", bk_d, KT)
            bk2_t = bias_tile("bk2_t", bk2_d, KT)
            bo_t = bias_tile("bo_t", bo_d, KT)
            b2_t = bias_tile("b2_t", b2_d, KT)
            b1_t = bias_tile("b1_t", b1_d, FT)
            wsc_t = perm.tile([1, 2], f32, name="wsc_t")
            nc.sync.dma_start(wsc_t[:], wsc_d[:])
            wsc_b = perm.tile([64, 2], f32, name="wsc_b")
            nc.gpsimd.partition_broadcast(wsc_b[:], wsc_t[:])

            # bias rows for natural-layout V, broadcast to 128 partitions
            # via a K=1 ones matmul
            bv_rows = []
            with tc.tile_pool(name="bvp", bufs=1, space="PSUM") as bvp:
                for nm, dram in (("bv", bvrow_d), ("bv2", bv2row_d)):
                    row = perm.tile([1, D], bf16, name=f"{nm}row")
                    nc.sync.dma_start(row[:], dram[:])
                    ps = bvp.tile([P, D], f32, name=f"{nm}ps", tag="bvps")
                    for sl in (slice(0, 512), slice(512, 768)):
                        nc.tensor.matmul(ps[:, sl], ones_row[:], row[:, sl],
                                         start=True, stop=True)
                    bbx = perm.tile([P, D], bf16, name=f"{nm}bcast")
                    nc.vector.tensor_copy(bbx[:], ps[:])
                    bv_rows.append(bbx)
            bv_b, bv2_b = bv_rows

            # ---- load inputs ----
            aT = []
            for kt in range(KT):
                t = res_pool.tile([P, S], f32, name=f"aT{kt}", tag="res")
                nc.sync.dma_start(t[:], aT_d[kt * P:(kt + 1) * P, :])
                aT.append(t)
            oT = []
            for kt in range(KT):
                t = oT_pool.tile([P, S], bf16, name=f"oT{kt}", tag="oT")
                nc.sync.dma_start(t[:], oT_d[kt * P:(kt + 1) * P, :])
                oT.append(t)

            # ---- layer norm (transposed layout) ----
            # src: 6 [128,1024] SBUF tiles (f32 or bf16)
            def ln_T(src, src_is_bf, ln_psum, name):
                tbf, tsq = [], []
                for kt in range(KT):
                    if src_is_bf:
                        c = src[kt]
                    else:
                        c = bbt(f"{name}bf{kt}")
                        nc.vector.tensor_copy(c[:], src[kt][:])
                    q = bbt(f"{name}sq{kt}")
                    nc.vector.tensor_tensor(q[:], src[kt][:], src[kt][:], Ax.mult)
                    tbf.append(c)
                    tsq.append(q)
                msum = ln_psum.tile([P, S], f32, name=f"{name}ms", tag="lnms")
                sqsum = ln_psum.tile([P, S], f32, name=f"{name}vs", tag="lnvs")
                for kt in range(KT):
                    st, sp = kt == 0, kt == KT - 1
                    for nh in range(2):
                        sl = slice(nh * 512, (nh + 1) * 512)
                        nc.tensor.matmul(msum[:, sl], ones_kk[:], tbf[kt][:, sl],
                                         start=st, stop=sp)
                        nc.tensor.matmul(sqsum[:, sl], ones_kk[:], tsq[kt][:, sl],
                                         start=st, stop=sp)
                m_s = fwt(f"{name}m")
                nc.vector.tensor_scalar(m_s[:], msum[:], 1.0 / D, None, Ax.mult)
                m2 = fwt(f"{name}m2")
                nc.vector.tensor_tensor(m2[:], m_s[:], m_s[:], Ax.mult)
                var = fwt(f"{name}var")
                nc.vector.scalar_tensor_tensor(var[:], sqsum[:], 1.0 / D, m2[:],
                                               Ax.mult, Ax.subtract)
                std = fwt(f"{name}std")
                nc.scalar.activation(std[:], var[:], Act.Sqrt, bias=eps_t[:])
                rstd = fwt(f"{name}rstd")
                act_recip(rstd[:], std[:])
                xn = []
                for kt in range(KT):
                    cen = fwt(f"{name}cen{kt}")
                    nc.vector.scalar_tensor_tensor(cen[:], msum[:], -1.0 / D,
                                                   src[kt][:], Ax.mult, Ax.add)
                    x = bbt(f"{name}xn{kt}")
                    nc.vector.tensor_tensor(x[:], cen[:], rstd[:], Ax.mult)
                    xn.append(x)
                return xn

            with tc.tile_pool(name="lnps_a", bufs=2, space="PSUM") as lnps:
                xnA = ln_T(aT, False, lnps, "A")
                xnO = ln_T(oT, True, lnps, "O")

            # ---- projections ----
            # q is stored per-head zero-padded to full 128 partitions so the
            # score matmul can contract over K=128 (the other head's k rows
            # multiply zero q rows). Keeps the PE array fully lit -> HAM stays
            # at the 2.4 GHz clock.
            qP = [bbt(f"qP{h}") for h in range(NH)]
            kTt = [bbt(f"kT{m}") for m in range(KT)]
            k2T = [bbt(f"k2T{m}") for m in range(KT)]
            # v buffers are 65-strided per head ([v(64) | ones(1)] x 12) with a
            # zeroed tail so the context matmul can take a full 128-wide lhsT
            # slice (rows 65..127 of its PSUM output are ignored).
            vN = [bbt(f"vN{j}") for j in range(JT)]
            v2N = [bbt(f"v2N{j}") for j in range(JT)]

            def load_w(name, kt, cols=None):
                t = wpool.tile([P, D], bf16, name=f"{name}w{kt}", tag="w")
                src = w_d[name]
                if cols is None:
                    nc.sync.dma_start(t[:], src[kt * P:(kt + 1) * P, :])
                else:
                    nc.sync.dma_start(t[:], src[kt * P:(kt + 1) * P,
                                                cols * D:(cols + 1) * D])
                return t

            with tc.tile_pool(name="projps", bufs=3, space="PSUM") as pps:
                # transposed-output projections: q, k, k2
                for pname, xn, bias, dst in (("wq", xnA, bq_t, None),
                                             ("wk", xnA, bk_t, kTt),
                                             ("wk2", xnO, bk2_t, k2T)):
                    wt = [load_w(pname, kt) for kt in range(KT)]
                    for mt in range(KT):
                        ps = pps.tile([P, S], f32, name=f"{pname}ps{mt}", tag="pps")
                        for kt in range(KT):
                            st, sp = kt == 0, kt == KT - 1
                            for nh in range(2):
                                sl = slice(nh * 512, (nh + 1) * 512)
                                nc.tensor.matmul(
                                    ps[:, sl],
                                    wt[kt][:, mt * P:(mt + 1) * P],
                                    xn[kt][:, sl], start=st, stop=sp)
                        if dst is not None:
                            nc.vector.tensor_scalar(dst[mt][:], ps[:],
                                                    bias[:, mt:mt + 1], None,
                                                    Ax.add)
                        else:
                            for hh in range(2):
                                h, po = 2 * mt + hh, hh * 64
                                t = qP[h]
                                nc.gpsimd.memset(t[:], 0.0)
                                nc.vector.tensor_scalar(
                                    t[po:po + 64, :], ps[po:po + 64, :],
                                    bias[po:po + 64, mt:mt + 1], None, Ax.add)
                # natural-layout projections with ones column: v, v2
                for pname, xn, bcast, dst in (("wv", xnA, bv_b, vN),
                                              ("wv2", xnO, bv2_b, v2N)):
                    wt = [load_w(pname, kt) for kt in range(KT)]
                    for jt in range(JT):
                        ps = pps.tile([P, D], f32, name=f"{pname}ps{jt}", tag="pps")
                        for kt in range(KT):
                            st, sp = kt == 0, kt == KT - 1
                            for sl in (slice(0, 512), slice(512, 768)):
                                nc.tensor.matmul(
                                    ps[:, sl],
                                    xn[kt][:, jt * P:(jt + 1) * P],
                                    wt[kt][:, sl], start=st, stop=sp)
                        nc.gpsimd.memset(dst[jt][:], 0.0)
                        dst_v = dst[jt][:, 0:NH * 80].rearrange(
                            "p (h c) -> p h c", c=80)
                        nc.vector.tensor_tensor(
                            dst_v[:, :, 0:64],
                            ps.rearrange("p (h c) -> p h c", c=64)[:],
                            bcast.rearrange("p (h c) -> p h c", c=64)[:],
                            Ax.add)
                        nc.gpsimd.memset(dst_v[:, :, 64:65], 1.0)

            if dbg:
                nc.sync.dma_start(dbg["dbgq"][:], qP[0][:])
                nc.sync.dma_start(dbg["dbgk"][:], kTt[0][:])
                nc.sync.dma_start(dbg["dbgv"][:], vN[0][:])

            # ---- attention ----
            wo_t = [load_w("wo", kt) for kt in range(KT)]
            ctx_t = [bbt(f"ctx{t}") for t in range(KT)]
            with tc.tile_pool(name="attnps", bufs=2, space="PSUM") as aps:
                for h in range(NH):
                    td, po = h // 2, (h % 2) * 64
                    tmps = []
                    for typ, (kk, vv) in enumerate(((kTt, vN), (k2T, v2N))):
                        cacc = aps.tile([P, S], f32, name=f"cv{h}_{typ}", tag="cv")
                        for jt in range(JT):
                            sT = aps.tile([P, S], f32, name=f"sT{h}_{typ}_{jt}",
                                          tag="sT")
                            for nh in range(2):
                                sl = slice(nh * 512, (nh + 1) * 512)
                                nc.tensor.matmul(
                                    sT[:, sl],
                                    kk[td][:, jt * P:(jt + 1) * P],
                                    qP[h][:, sl],
                                    start=True, stop=True)
                            pT = bbt(f"pT{h}_{typ}_{jt}")
                            for nh in range(2):
                                sl = slice(nh * 512, (nh + 1) * 512)
                                nc.scalar.activation(pT[:, sl], sT[:, sl], Act.Exp)
                            if dbg and h == 0 and typ == 0 and jt == 0:
                                nc.sync.dma_start(dbg["dbgp"][:], pT[:])
                            for nh in range(2):
                                sl = slice(nh * 512, (nh + 1) * 512)
                                nc.tensor.matmul(
                                    cacc[:, sl],
                                    vv[jt][:, h * 80:h * 80 + P],
                                    pT[:, sl],
                                    start=(jt == 0), stop=(jt == JT - 1))
                        recip = fwt(f"rc{h}_{typ}", (1, S))
                        nc.vector.reciprocal(recip[:], cacc[64:65, :])
                        rb = fwt(f"rb{h}_{typ}", (64, S))
                        nc.gpsimd.partition_broadcast(rb[:], recip[:])
                        tmp = bbt(f"tm{h}_{typ}", (64, S))
                        nc.vector.scalar_tensor_tensor(
                            tmp[:], cacc[0:64, :], wsc_b[0:64, typ:typ + 1],
                            rb[:], Ax.mult, Ax.mult)
                        tmps.append(tmp)
                    nc.vector.tensor_add(ctx_t[td][po:po + 64, :],
                                         tmps[0][:], tmps[1][:])
                    if dbg and h == 1:
                        nc.sync.dma_start(dbg["dbgc"][:], ctx_t[0][:])

            # ---- out-projection + residual ----
            x1 = []
            with tc.tile_pool(name="opps", bufs=2, space="PSUM") as ops:
                for mt in range(KT):
                    ps = ops.tile([P, S], f32, name=f"ops{mt}", tag="ops")
                    for kt in range(KT):
                        st, sp = kt == 0, kt == KT - 1
                        for nh in range(2):
                            sl = slice(nh * 512, (nh + 1) * 512)
                            nc.tensor.matmul(ps[:, sl],
                                             wo_t[kt][:, mt * P:(mt + 1) * P],
                                             ctx_t[kt][:, sl], start=st, stop=sp)
                    t = res_pool.tile([P, S], f32, name=f"x1_{mt}", tag="res")
                    nc.vector.scalar_tensor_tensor(t[:], ps[:], bo_t[:, mt:mt + 1],
                                                   aT[mt][:], Ax.add, Ax.add)
                    x1.append(t)

            # ---- LN2 ----
            with tc.tile_pool(name="lnps_b", bufs=1, space="PSUM") as lnps2:
                xn2 = ln_T(x1, False, lnps2, "B")

            # ---- MLP ----
            with tc.tile_pool(name="mlpps", bufs=3, space="PSUM") as mps:
                hbf = []
                for fq in range(4):
                    w1t = [load_w("w1", kt, cols=fq) for kt in range(KT)]
                    for fl in range(KT):
                        ft = fq * KT + fl
                        ps = mps.tile([P, S], f32, name=f"h_ps{ft}", tag="mps")
                        for kt in range(KT):
                            st, sp = kt == 0, kt == KT - 1
                            for nh in range(2):
                                sl = slice(nh * 512, (nh + 1) * 512)
                                nc.tensor.matmul(ps[:, sl],
                                                 w1t[kt][:, fl * P:(fl + 1) * P],
                                                 xn2[kt][:, sl], start=st, stop=sp)
                        hb = bbt(f"hbf{ft}")
                        nc.scalar.activation(hb[:], ps[:], Act.Gelu_apprx_tanh,
                                             bias=b1_t[:, ft:ft + 1])
                        hbf.append(hb)
                for half in range(2):
                    psl = []
                    for ml in range(3):
                        ps = mps.tile([P, S], f32, name=f"o_ps{half}_{ml}",
                                      tag="mps")
                        psl.append(ps)
                    for kt in range(FT):
                        w2t = load_w("w2", kt)
                        for ml in range(3):
                            mt = half * 3 + ml
                            st, sp = kt == 0, kt == FT - 1
                            for nh in range(2):
                                sl = slice(nh * 512, (nh + 1) * 512)
                                nc.tensor.matmul(psl[ml][:, sl],
                                                 w2t[:, mt * P:(mt + 1) * P],
                                                 hbf[kt][:, sl], start=st, stop=sp)
                    for ml in range(3):
                        mt = half * 3 + ml
                        ot = fwt(f"out{mt}")
                        nc.vector.scalar_tensor_tensor(ot[:], psl[ml][:],
                                                       b2_t[:, mt:mt + 1],
                                                       x1[mt][:], Ax.add, Ax.add)
                        nc.sync.dma_start(out_d[mt * P:(mt + 1) * P, :], ot[:])

    nc.compile()
    return nc


def _get_program():
    global _PROGRAM
    if _PROGRAM is None:
        _PROGRAM = _build_program()
    return _PROGRAM


def _fold_core(inp, b, s):
    """Host-side shard + weight folding for core (batch b, stream s)."""
    if s == 0:
        a, o = inp['x'][b], inp['y'][b]
        g1s, b1s, g1o, b1o = inp['ln1x_g'], inp['ln1x_b'], inp['ln1y_g'], inp['ln1y_b']
        Wq, bq, Wk, bk, Wv, bv = inp['Wq'], inp['bq'], inp['Wk'], inp['bk'], inp['Wv'], inp['bv']
        Wk2, bk2, Wv2, bv2 = inp['Wkd'], inp['bkd'], inp['Wvd'], inp['bvd']
        Wo, bo = inp['Wo'], inp['bo']
        ws, wc = inp['w11'][0], inp['w12'][0]
        g2, b2g = inp['ln2x_g'], inp['ln2x_b']
        W1, b1, W2, b2 = inp['W1'], inp['b1'], inp['W2'], inp['b2']
    else:
        a, o = inp['y'][b], inp['x'][b]
        g1s, b1s, g1o, b1o = inp['ln1y_g'], inp['ln1y_b'], inp['ln1x_g'], inp['ln1x_b']
        Wq, bq, Wk, bk, Wv, bv = inp['Wqd'], inp['bqd'], inp['Wkd'], inp['bkd'], inp['Wvd'], inp['bvd']
        Wk2, bk2, Wv2, bv2 = inp['Wk'], inp['bk'], inp['Wv'], inp['bv']
        Wo, bo = inp['Wod'], inp['bod']
        ws, wc = inp['w21'][0], inp['w22'][0]
        g2, b2g = inp['ln2y_g'], inp['ln2y_b']
        W1, b1, W2, b2 = inp['W1d'], inp['b1d'], inp['W2d'], inp['b2d']

    m = {
        'aT': np.ascontiguousarray(a.T, np.float32),
        'oT': np.ascontiguousarray(o.T).astype(BF),
        'wq': np.ascontiguousarray(g1s[:, None] * Wq * SCALE).astype(BF),
        'wk': np.ascontiguousarray(g1s[:, None] * Wk).astype(BF),
        'wv': np.ascontiguousarray(g1s[:, None] * Wv).astype(BF),
        'wk2': np.ascontiguousarray(g1o[:, None] * Wk2).astype(BF),
        'wv2': np.ascontiguousarray(g1o[:, None] * Wv2).astype(BF),
        'wo': np.ascontiguousarray(Wo).astype(BF),
        'w1': np.ascontiguousarray(g2[:, None] * W1).astype(BF),
        'w2': np.ascontiguousarray(W2).astype(BF),
        'bq': (SCALE * (bq + b1s @ Wq)).astype(np.float32).reshape(D, 1),
        'bk': (bk + b1s @ Wk).astype(np.float32).reshape(D, 1),
        'bk2': (bk2 + b1o @ Wk2).astype(np.float32).reshape(D, 1),
        'bo': ((ws + wc) * bo).astype(np.float32).reshape(D, 1),
        'b2': np.asarray(b2, np.float32).reshape(D, 1),
        'b1': (b1 + b2g @ W1).astype(np.float32).reshape(F, 1),
        'bvrow': (bv + b1s @ Wv).astype(np.float32).reshape(1, D).astype(BF),
        'bv2row': (bv2 + b1o @ Wv2).astype(np.float32).reshape(1, D).astype(BF),
        'wsc': np.array([[ws, wc]], np.float32),
    }
    return m


LAST_RESULTS = None


def kernel(**inputs):
    from concourse.bass_utils import run_bass_kernel_spmd
    global LAST_RESULTS

    inp = {k: np.asarray(v, np.float32) for k, v in inputs.items()}
    B = inp['x'].shape[0]

    nc = _get_program()
    in_maps = [_fold_core(inp, core // 2, core % 2) for core in range(2 * B)]
    res = run_bass_kernel_spmd(
        nc, in_maps, core_ids=list(range(2 * B)),
        trace=bool(os.environ.get("KERNEL_TRACE")))
    LAST_RESULTS = res

    x_out = np.empty((B, S, D), np.float32)
    y_out = np.empty((B, S, D), np.float32)
    for b in range(B):
        x_out[b] = res.results[2 * b]["outT"].T
        y_out[b] = res.results[2 * b + 1]["outT"].T
    return (x_out, y_out)


# revision 26
# speedup vs baseline: 1.1060x; 1.1060x over previous
"""Trainium2 Bass kernel for the dual-stream transformer block
(nn_Block_87840671138274).

Sharding: 8 cores = 4 batches x 2 streams. Core i handles batch i//2,
stream i%2 (0=x, 1=y) and produces that stream's full output. Each core
redundantly computes the *other* stream's LN + K/V projections (~12%
extra FLOPs) so there are zero collectives.

Layout: everything transposed ([D, S] with D on SBUF partitions).
- Host pre-transposes inputs and pre-folds LN gamma/beta + softmax SCALE
  into the projection weights/biases (exact algebra, f32).
- LN stats (mean / mean-of-squares) via ones-matmul partition reductions.
- Attention computes s^T = k^T.T @ q^T per head (K=64), exp on ACT, and
  the softmax denominator comes from a ones-column appended to V
  (natural layout), accumulated in the same PSUM matmul as the context.
- No max-subtraction in softmax: scores are ~N(0, 0.31), |s| < ~2.
- bf16 matmuls with f32 PSUM accumulation; residual stream kept f32.
"""
import os
import numpy as np
import ml_dtypes

P = 128
S = 1024
D = 768
F = 3072
NH = 12
HD = 64
KT = D // P     # 6
JT = S // P     # 8
FT = F // P     # 24
EPS = 1e-6
SCALE = np.float32(1.0 / np.sqrt(HD))
BF = ml_dtypes.bfloat16

_PROGRAM = None


def _build_program():
    import concourse.bass as bass
    import concourse.bacc as bacc
    import concourse.tile as tile
    from concourse import mybir
    from contextlib import ExitStack

    f32 = mybir.dt.float32
    bf16 = mybir.dt.bfloat16
    Ax = mybir.AluOpType
    Act = mybir.ActivationFunctionType

    nc = bacc.Bacc("TRN2", target_bir_lowering=False, debug=False, num_devices=8)

    aT_d = nc.dram_tensor("aT", [D, S], f32, kind="ExternalInput").ap()
    oT_d = nc.dram_tensor("oT", [D, S], bf16, kind="ExternalInput").ap()
    w_d = {}
    for w in ("wq", "wk", "wv", "wk2", "wv2", "wo"):
        w_d[w] = nc.dram_tensor(w, [D, D], bf16, kind="ExternalInput").ap()
    w_d["w1"] = nc.dram_tensor("w1", [D, F], bf16, kind="ExternalInput").ap()
    w_d["w2"] = nc.dram_tensor("w2", [F, D], bf16, kind="ExternalInput").ap()
    bq_d = nc.dram_tensor("bq", [D, 1], f32, kind="ExternalInput").ap()
    bk_d = nc.dram_tensor("bk", [D, 1], f32, kind="ExternalInput").ap()
    bk2_d = nc.dram_tensor("bk2", [D, 1], f32, kind="ExternalInput").ap()
    bo_d = nc.dram_tensor("bo", [D, 1], f32, kind="ExternalInput").ap()
    b2_d = nc.dram_tensor("b2", [D, 1], f32, kind="ExternalInput").ap()
    b1_d = nc.dram_tensor("b1", [F, 1], f32, kind="ExternalInput").ap()
    bvrow_d = nc.dram_tensor("bvrow", [1, D], bf16, kind="ExternalInput").ap()
    bv2row_d = nc.dram_tensor("bv2row", [1, D], bf16, kind="ExternalInput").ap()
    wsc_d = nc.dram_tensor("wsc", [1, 2], f32, kind="ExternalInput").ap()
    out_d = nc.dram_tensor("outT", [D, S], f32, kind="ExternalOutput").ap()
    dbg = {}
    if os.environ.get("KERNEL_DEBUG"):
        for nm in ("dbgq", "dbgk", "dbgv", "dbgp", "dbgc"):
            dbg[nm] = nc.dram_tensor(nm, [P, S], bf16, kind="ExternalOutput").ap()

    with tile.TileContext(nc) as tc:
        with ExitStack() as ctx:
            perm = ctx.enter_context(tc.tile_pool(name="perm", bufs=1))
            res_pool = ctx.enter_context(tc.tile_pool(name="res", bufs=7))
            oT_pool = ctx.enter_context(tc.tile_pool(name="oTp", bufs=6))
            bb = ctx.enter_context(tc.tile_pool(name="bigbf", bufs=54))
            fw = ctx.enter_context(tc.tile_pool(name="f32w", bufs=7))
            wpool = ctx.enter_context(tc.tile_pool(name="wpool", bufs=12))

            def bbt(name, shape=(P, S)):
                return bb.tile(list(shape), bf16, name=name, tag="bb")

            def fwt(name, shape=(P, S)):
                return fw.tile(list(shape), f32, name=name, tag="fw")

            def act_recip(out_ap, in_ap):
                # Table-based reciprocal on the (idle) Scalar engine. The
                # nc.scalar.activation wrapper refuses Reciprocal for accuracy
                # reasons; here the inputs are well-conditioned positives
                # (softmax denominators ~1e3, LN std ~1) and table accuracy is
                # far inside this kernel's error budget -- and it takes ~1us
                # vs 6.5us for the exact DVE reciprocal, off the DVE critical
                # path.
                se = nc.scalar
                se.add_instruction(mybir.InstActivation(
                    name=nc.get_next_instruction_name(),
                    func=Act.Reciprocal,
                    ins=[se.lower_ap(in_ap),
                         mybir.ImmediateValue(dtype=f32, value=0.0),
                         mybir.ImmediateValue(dtype=f32, value=1.0),
                         mybir.ImmediateValue(dtype=f32, value=0.0)],
                    outs=[se.lower_ap(out_ap)]))

            ones_kk = perm.tile([P, P], bf16, name="ones_kk")
            nc.gpsimd.memset(ones_kk[:], 1.0)
            ones_row = perm.tile([1, P], bf16, name="ones_row")
            nc.gpsimd.memset(ones_row[:], 1.0)
            eps_t = perm.tile([P, 1], f32, name="eps_t")
            nc.gpsimd.memset(eps_t[:], float(EPS))

            def bias_tile(name, dram, nt):
                t = perm.tile([P, nt], f32, name=name)
                nc.sync.dma_start(t[:], dram.rearrange("(t p) o -> p (t o)", p=P))
                return t

            bq_t = bias_tile("bq_t", bq_d, KT)
            bk_t = bias_tile("bk_t", bk_d, KT)
            bk2_t = bias_tile("bk2_t", bk2_d, KT)
            bo_t = bias_tile("bo_t", bo_d, KT)
            b2_t = bias_tile("b2_t", b2_d, KT)
            b1_t = bias_tile("b1_t", b1_d, FT)
            wsc_t = perm.tile([1, 2], f32, name="wsc_t")
            nc.sync.dma_start(wsc_t[:], wsc_d[:])
            wsc_b = perm.tile([64, 2], f32, name="wsc_b")
            nc.gpsimd.partition_broadcast(wsc_b[:], wsc_t[:])

            # bias rows for natural-layout V, broadcast to 128 partitions
            # via a K=1 ones matmul
            bv_rows = []
            with tc.tile_pool(name="bvp", bufs=1, space="PSUM") as bvp:
                for nm, dram in (("bv", bvrow_d), ("bv2", bv2row_d)):
                    row = perm.tile([1, D], bf16, name=f"{nm}row")
                    nc.sync.dma_start(row[:], dram[:])
                    ps = bvp.tile([P, D], f32, name=f"{nm}ps", tag="bvps")
                    for sl in (slice(0, 512), slice(512, 768)):
                        nc.tensor.matmul(ps[:, sl], ones_row[:], row[:, sl],
                                         start=True, stop=True)
                    bbx = perm.tile([P, D], bf16, name=f"{nm}bcast")
                    nc.vector.tensor_copy(bbx[:], ps[:])
                    bv_rows.append(bbx)
            bv_b, bv2_b = bv_rows

            # ---- load inputs ----
            aT = []
            for kt in range(KT):
                t = res_pool.tile([P, S], f32, name=f"aT{kt}", tag="res")
                nc.sync.dma_start(t[:], aT_d[kt * P:(kt + 1) * P, :])
                aT.append(t)
            oT = []
            for kt in range(KT):
                t = oT_pool.tile([P, S], bf16, name=f"oT{kt}", tag="oT")
                nc.sync.dma_start(t[:], oT_d[kt * P:(kt + 1) * P, :])
                oT.append(t)

            # ---- layer norm (transposed layout) ----
            # src: 6 [128,1024] SBUF tiles (f32 or bf16)
            def ln_T(src, src_is_bf, ln_psum, name):
                tbf, tsq = [], []
                for kt in range(KT):
                    if src_is_bf:
                        c = src[kt]
                    else:
                        c = bbt(f"{name}bf{kt}")
                        nc.vector.tensor_copy(c[:], src[kt][:])
                    q = bbt(f"{name}sq{kt}")
                    nc.vector.tensor_tensor(q[:], src[kt][:], src[kt][:], Ax.mult)
                    tbf.append(c)
                    tsq.append(q)
                msum = ln_psum.tile([P, S], f32, name=f"{name}ms", tag="lnms")
                sqsum = ln_psum.tile([P, S], f32, name=f"{name}vs", tag="lnvs")
                for kt in range(KT):
                    st, sp = kt == 0, kt == KT - 1
                    for nh in range(2):
                        sl = slice(nh * 512, (nh + 1) * 512)
                        nc.tensor.matmul(msum[:, sl], ones_kk[:], tbf[kt][:, sl],
                                         start=st, stop=sp)
                        nc.tensor.matmul(sqsum[:, sl], ones_kk[:], tsq[kt][:, sl],
                                         start=st, stop=sp)
                m_s = fwt(f"{name}m")
                nc.vector.tensor_scalar(m_s[:], msum[:], 1.0 / D, None, Ax.mult)
                m2 = fwt(f"{name}m2")
                nc.vector.tensor_tensor(m2[:], m_s[:], m_s[:], Ax.mult)
                var = fwt(f"{name}var")
                nc.vector.scalar_tensor_tensor(var[:], sqsum[:], 1.0 / D, m2[:],
                                               Ax.mult, Ax.subtract)
                std = fwt(f"{name}std")
                nc.scalar.activation(std[:], var[:], Act.Sqrt, bias=eps_t[:])
                rstd = fwt(f"{name}rstd")
                act_recip(rstd[:], std[:])
                xn = []
                for kt in range(KT):
                    cen = fwt(f"{name}cen{kt}")
                    nc.vector.scalar_tensor_tensor(cen[:], msum[:], -1.0 / D,
                                                   src[kt][:], Ax.mult, Ax.add)
                    x = bbt(f"{name}xn{kt}")
                    nc.vector.tensor_tensor(x[:], cen[:], rstd[:], Ax.mult)
                    xn.append(x)
                return xn

            with tc.tile_pool(name="lnps_a", bufs=2, space="PSUM") as lnps:
                xnA = ln_T(aT, False, lnps, "A")
                xnO = ln_T(oT, True, lnps, "O")

            # ---- projections ----
            # q is stored per-head zero-padded to full 128 partitions so the
            # score matmul can contract over K=128 (the other head's k rows
            # multiply zero q rows). Keeps the PE array fully lit -> HAM stays
            # at the 2.4 GHz clock.
            qP = [bbt(f"qP{h}") for h in range(NH)]
            kTt = [bbt(f"kT{m}") for m in range(KT)]
            k2T = [bbt(f"k2T{m}") for m in range(KT)]
            # v buffers are 65-strided per head ([v(64) | ones(1)] x 12) with a
            # zeroed tail so the context matmul can take a full 128-wide lhsT
            # slice (rows 65..127 of its PSUM output are ignored).
            vN = [bbt(f"vN{j}") for j in range(JT)]
            v2N = [bbt(f"v2N{j}") for j in range(JT)]

            def load_w(name, kt, cols=None):
                t = wpool.tile([P, D], bf16, name=f"{name}w{kt}", tag="w")
                src = w_d[name]
                if cols is None:
                    nc.sync.dma_start(t[:], src[kt * P:(kt + 1) * P, :])
                else:
                    nc.sync.dma_start(t[:], src[kt * P:(kt + 1) * P,
                                                cols * D:(cols + 1) * D])
                return t

            with tc.tile_pool(name="projps", bufs=3, space="PSUM") as pps:
                # transposed-output projections: q, k, k2
                for pname, xn, bias, dst in (("wq", xnA, bq_t, None),
                                             ("wk", xnA, bk_t, kTt),
                                             ("wk2", xnO, bk2_t, k2T)):
                    wt = [load_w(pname, kt) for kt in range(KT)]
                    for mt in range(KT):
                        ps = pps.tile([P, S], f32, name=f"{pname}ps{mt}", tag="pps")
                        for kt in range(KT):
                            st, sp = kt == 0, kt == KT - 1
                            for nh in range(2):
                                sl = slice(nh * 512, (nh + 1) * 512)
                                nc.tensor.matmul(
                                    ps[:, sl],
                                    wt[kt][:, mt * P:(mt + 1) * P],
                                    xn[kt][:, sl], start=st, stop=sp)
                        if dst is not None:
                            nc.vector.tensor_scalar(dst[mt][:], ps[:],
                                                    bias[:, mt:mt + 1], None,
                                                    Ax.add)
                        else:
                            for hh in range(2):
                                h, po = 2 * mt + hh, hh * 64
                                t = qP[h]
                                nc.gpsimd.memset(t[:], 0.0)
                                nc.vector.tensor_scalar(
                                    t[po:po + 64, :], ps[po:po + 64, :],
                                    bias[po:po + 64, mt:mt + 1], None, Ax.add)
                # natural-layout projections with ones column: v, v2
                for pname, xn, bcast, dst in (("wv", xnA, bv_b, vN),
                                              ("wv2", xnO, bv2_b, v2N)):
                    wt = [load_w(pname, kt) for kt in range(KT)]
                    for jt in range(JT):
                        ps = pps.tile([P, D], f32, name=f"{pname}ps{jt}", tag="pps")
                        for kt in range(KT):
                            st, sp = kt == 0, kt == KT - 1
                            for sl in (slice(0, 512), slice(512, 768)):
                                nc.tensor.matmul(
                                    ps[:, sl],
                                    xn[kt][:, jt * P:(jt + 1) * P],
                                    wt[kt][:, sl], start=st, stop=sp)
                        nc.gpsimd.memset(dst[jt][:], 0.0)
                        dst_v = dst[jt][:, 0:NH * 80].rearrange(
                            "p (h c) -> p h c", c=80)
                        nc.vector.tensor_tensor(
                            dst_v[:, :, 0:64],
                            ps.rearrange("p (h c) -> p h c", c=64)[:],
                            bcast.rearrange("p (h c) -> p h c", c=64)[:],
                            Ax.add)
                        nc.gpsimd.memset(dst_v[:, :, 64:65], 1.0)

            if dbg:
                nc.sync.dma_start(dbg["dbgq"][:], qP[0][:])
                nc.sync.dma_start(dbg["dbgk"][:], kTt[0][:])
                nc.sync.dma_start(dbg["dbgv"][:], vN[0][:])

            # ---- attention ----
            wo_t = [load_w("wo", kt) for kt in range(KT)]
            ctx_t = [bbt(f"ctx{t}") for t in range(KT)]
            with tc.tile_pool(name="attnps", bufs=2, space="PSUM") as aps:
                for h in range(NH):
                    td, po = h // 2, (h % 2) * 64
                    tmps = []
                    for typ, (kk, vv) in enumerate(((kTt, vN), (k2T, v2N))):
                        cacc = aps.tile([P, S], f32, name=f"cv{h}_{typ}", tag="cv")
                        for jt in range(JT):
                            sT = aps.tile([P, S], f32, name=f"sT{h}_{typ}_{jt}",
                                          tag="sT")
                            for nh in range(2):
                                sl = slice(nh * 512, (nh + 1) * 512)
                                nc.tensor.matmul(
                                    sT[:, sl],
                                    kk[td][:, jt * P:(jt + 1) * P],
                                    qP[h][:, sl],
                                    start=True, stop=True)
                            pT = bbt(f"pT{h}_{typ}_{jt}")
                            nc.scalar.activation(pT[:], sT[:], Act.Exp)
                            if dbg and h == 0 and typ == 0 and jt == 0:
                                nc.sync.dma_start(dbg["dbgp"][:], pT[:])
                            for nh in range(2):
                                sl = slice(nh * 512, (nh + 1) * 512)
                                nc.tensor.matmul(
                                    cacc[:, sl],
                                    vv[jt][:, h * 80:h * 80 + P],
                                    pT[:, sl],
                                    start=(jt == 0), stop=(jt == JT - 1))
                        recip = fwt(f"rc{h}_{typ}", (1, S))
                        nc.vector.reciprocal(recip[:], cacc[64:65, :])
                        rb = fwt(f"rb{h}_{typ}", (64, S))
                        nc.gpsimd.partition_broadcast(rb[:], recip[:])
                        tmp = bbt(f"tm{h}_{typ}", (64, S))
                        nc.vector.scalar_tensor_tensor(
                            tmp[:], cacc[0:64, :], wsc_b[0:64, typ:typ + 1],
                            rb[:], Ax.mult, Ax.mult)
                        tmps.append(tmp)
                    nc.vector.tensor_add(ctx_t[td][po:po + 64, :],
                                         tmps[0][:], tmps[1][:])
                    if dbg and h == 1:
                        nc.sync.dma_start(dbg["dbgc"][:], ctx_t[0][:])

            # ---- out-projection + residual ----
            x1 = []
            with tc.tile_pool(name="opps", bufs=2, space="PSUM") as ops:
                for mt in range(KT):
                    ps = ops.tile([P, S], f32, name=f"ops{mt}", tag="ops")
                    for kt in range(KT):
                        st, sp = kt == 0, kt == KT - 1
                        for nh in range(2):
                            sl = slice(nh * 512, (nh + 1) * 512)
                            nc.tensor.matmul(ps[:, sl],
                                             wo_t[kt][:, mt * P:(mt + 1) * P],
                                             ctx_t[kt][:, sl], start=st, stop=sp)
                    t = res_pool.tile([P, S], f32, name=f"x1_{mt}", tag="res")
                    nc.vector.scalar_tensor_tensor(t[:], ps[:], bo_t[:, mt:mt + 1],
                                                   aT[mt][:], Ax.add, Ax.add)
                    x1.append(t)

            # ---- LN2 ----
            with tc.tile_pool(name="lnps_b", bufs=1, space="PSUM") as lnps2:
                xn2 = ln_T(x1, False, lnps2, "B")

            # ---- MLP ----
            with tc.tile_pool(name="mlpps", bufs=3, space="PSUM") as mps:
                hbf = []
                for fq in range(4):
                    w1t = [load_w("w1", kt, cols=fq) for kt in range(KT)]
                    for fl in range(KT):
                        ft = fq * KT + fl
                        ps = mps.tile([P, S], f32, name=f"h_ps{ft}", tag="mps")
                        for kt in range(KT):
                            st, sp = kt == 0, kt == KT - 1
                            for nh in range(2):
                                sl = slice(nh * 512, (nh + 1) * 512)
                                nc.tensor.matmul(ps[:, sl],
                                                 w1t[kt][:, fl * P:(fl + 1) * P],
                                                 xn2[kt][:, sl], start=st, stop=sp)
                        hb = bbt(f"hbf{ft}")
                        nc.scalar.activation(hb[:], ps[:], Act.Gelu_apprx_tanh,
                                             bias=b1_t[:, ft:ft + 1])
                        hbf.append(hb)
                for half in range(2):
                    psl = []
                    for ml in range(3):
                        ps = mps.tile([P, S], f32, name=f"o_ps{half}_{ml}",
                                      tag="mps")
                        psl.append(ps)
                    for kt in range(FT):
                        w2t = load_w("w2", kt)
                        for ml in range(3):
                            mt = half * 3 + ml
                            st, sp = kt == 0, kt == FT - 1
                            for nh in range(2):
                                sl = slice(nh * 512, (nh + 1) * 512)
                                nc.tensor.matmul(psl[ml][:, sl],
                                                 w2t[:, mt * P:(mt + 1) * P],
                                                 hbf[kt][:, sl], start=st, stop=sp)
                    for ml in range(3):
                        mt = half * 3 + ml
                        ot = fwt(f"out{mt}")
                        nc.vector.scalar_tensor_tensor(ot[:], psl[ml][:],
                                                       b2_t[:, mt:mt + 1],
                                                       x1[mt][:], Ax.add, Ax.add)
                        nc.sync.dma_start(out_d[mt * P:(mt + 1) * P, :], ot[:])

    nc.compile()
    return nc


def _get_program():
    global _PROGRAM
    if _PROGRAM is None:
        _PROGRAM = _build_program()
    return _PROGRAM


def _fold_core(inp, b, s):
    """Host-side shard + weight folding for core (batch b, stream s)."""
    if s == 0:
        a, o = inp['x'][b], inp['y'][b]
        g1s, b1s, g1o, b1o = inp['ln1x_g'], inp['ln1x_b'], inp['ln1y_g'], inp['ln1y_b']
        Wq, bq, Wk, bk, Wv, bv = inp['Wq'], inp['bq'], inp['Wk'], inp['bk'], inp['Wv'], inp['bv']
        Wk2, bk2, Wv2, bv2 = inp['Wkd'], inp['bkd'], inp['Wvd'], inp['bvd']
        Wo, bo = inp['Wo'], inp['bo']
        ws, wc = inp['w11'][0], inp['w12'][0]
        g2, b2g = inp['ln2x_g'], inp['ln2x_b']
        W1, b1, W2, b2 = inp['W1'], inp['b1'], inp['W2'], inp['b2']
    else:
        a, o = inp['y'][b], inp['x'][b]
        g1s, b1s, g1o, b1o = inp['ln1y_g'], inp['ln1y_b'], inp['ln1x_g'], inp['ln1x_b']
        Wq, bq, Wk, bk, Wv, bv = inp['Wqd'], inp['bqd'], inp['Wkd'], inp['bkd'], inp['Wvd'], inp['bvd']
        Wk2, bk2, Wv2, bv2 = inp['Wk'], inp['bk'], inp['Wv'], inp['bv']
        Wo, bo = inp['Wod'], inp['bod']
        ws, wc = inp['w21'][0], inp['w22'][0]
        g2, b2g = inp['ln2y_g'], inp['ln2y_b']
        W1, b1, W2, b2 = inp['W1d'], inp['b1d'], inp['W2d'], inp['b2d']

    m = {
        'aT': np.ascontiguousarray(a.T, np.float32),
        'oT': np.ascontiguousarray(o.T).astype(BF),
        'wq': np.ascontiguousarray(g1s[:, None] * Wq * SCALE).astype(BF),
        'wk': np.ascontiguousarray(g1s[:, None] * Wk).astype(BF),
        'wv': np.ascontiguousarray(g1s[:, None] * Wv).astype(BF),
        'wk2': np.ascontiguousarray(g1o[:, None] * Wk2).astype(BF),
        'wv2': np.ascontiguousarray(g1o[:, None] * Wv2).astype(BF),
        'wo': np.ascontiguousarray(Wo).astype(BF),
        'w1': np.ascontiguousarray(g2[:, None] * W1).astype(BF),
        'w2': np.ascontiguousarray(W2).astype(BF),
        'bq': (SCALE * (bq + b1s @ Wq)).astype(np.float32).reshape(D, 1),
        'bk': (bk + b1s @ Wk).astype(np.float32).reshape(D, 1),
        'bk2': (bk2 + b1o @ Wk2).astype(np.float32).reshape(D, 1),
        'bo': ((ws + wc) * bo).astype(np.float32).reshape(D, 1),
        'b2': np.asarray(b2, np.float32).reshape(D, 1),
        'b1': (b1 + b2g @ W1).astype(np.float32).reshape(F, 1),
        'bvrow': (bv + b1s @ Wv).astype(np.float32).reshape(1, D).astype(BF),
        'bv2row': (bv2 + b1o @ Wv2).astype(np.float32).reshape(1, D).astype(BF),
        'wsc': np.array([[ws, wc]], np.float32),
    }
    return m


LAST_RESULTS = None


def kernel(**inputs):
    from concourse.bass_utils import run_bass_kernel_spmd
    global LAST_RESULTS

    inp = {k: np.asarray(v, np.float32) for k, v in inputs.items()}
    B = inp['x'].shape[0]

    nc = _get_program()
    in_maps = [_fold_core(inp, core // 2, core % 2) for core in range(2 * B)]
    res = run_bass_kernel_spmd(
        nc, in_maps, core_ids=list(range(2 * B)),
        trace=bool(os.environ.get("KERNEL_TRACE")))
    LAST_RESULTS = res

    x_out = np.empty((B, S, D), np.float32)
    y_out = np.empty((B, S, D), np.float32)
    for b in range(B):
        x_out[b] = res.results[2 * b]["outT"].T
        y_out[b] = res.results[2 * b + 1]["outT"].T
    return (x_out, y_out)


# revision 27
# speedup vs baseline: 1.1087x; 1.0025x over previous
"""Trainium2 Bass kernel for the dual-stream transformer block
(nn_Block_87840671138274).

Sharding: 8 cores = 4 batches x 2 streams. Core i handles batch i//2,
stream i%2 (0=x, 1=y) and produces that stream's full output. Each core
redundantly computes the *other* stream's LN + K/V projections (~12%
extra FLOPs) so there are zero collectives.

Layout: everything transposed ([D, S] with D on SBUF partitions).
- Host pre-transposes inputs and pre-folds LN gamma/beta + softmax SCALE
  into the projection weights/biases (exact algebra, f32).
- LN stats (mean / mean-of-squares) via ones-matmul partition reductions.
- Attention computes s^T = k^T.T @ q^T per head (K=64), exp on ACT, and
  the softmax denominator comes from a ones-column appended to V
  (natural layout), accumulated in the same PSUM matmul as the context.
- No max-subtraction in softmax: scores are ~N(0, 0.31), |s| < ~2.
- bf16 matmuls with f32 PSUM accumulation; residual stream kept f32.
"""
import os
import numpy as np
import ml_dtypes

P = 128
S = 1024
D = 768
F = 3072
NH = 12
HD = 64
KT = D // P     # 6
JT = S // P     # 8
FT = F // P     # 24
EPS = 1e-6
SCALE = np.float32(1.0 / np.sqrt(HD))
BF = ml_dtypes.bfloat16

_PROGRAM = None


def _build_program():
    import concourse.bass as bass
    import concourse.bacc as bacc
    import concourse.tile as tile
    from concourse import mybir
    from contextlib import ExitStack

    f32 = mybir.dt.float32
    bf16 = mybir.dt.bfloat16
    Ax = mybir.AluOpType
    Act = mybir.ActivationFunctionType

    nc = bacc.Bacc("TRN2", target_bir_lowering=False, debug=False, num_devices=8)

    aT_d = nc.dram_tensor("aT", [D, S], f32, kind="ExternalInput").ap()
    aTb_d = nc.dram_tensor("aTb", [D, S], bf16, kind="ExternalInput").ap()
    oT_d = nc.dram_tensor("oT", [D, S], bf16, kind="ExternalInput").ap()
    w_d = {}
    for w in ("wq", "wk", "wv", "wk2", "wv2", "wo"):
        w_d[w] = nc.dram_tensor(w, [D, D], bf16, kind="ExternalInput").ap()
    w_d["w1"] = nc.dram_tensor("w1", [D, F], bf16, kind="ExternalInput").ap()
    w_d["w2"] = nc.dram_tensor("w2", [F, D], bf16, kind="ExternalInput").ap()
    bq_d = nc.dram_tensor("bq", [D, 1], f32, kind="ExternalInput").ap()
    bk_d = nc.dram_tensor("bk", [D, 1], f32, kind="ExternalInput").ap()
    bk2_d = nc.dram_tensor("bk2", [D, 1], f32, kind="ExternalInput").ap()
    bo_d = nc.dram_tensor("bo", [D, 1], f32, kind="ExternalInput").ap()
    b2_d = nc.dram_tensor("b2", [D, 1], f32, kind="ExternalInput").ap()
    b1_d = nc.dram_tensor("b1", [F, 1], f32, kind="ExternalInput").ap()
    bvrow_d = nc.dram_tensor("bvrow", [1, D], bf16, kind="ExternalInput").ap()
    bv2row_d = nc.dram_tensor("bv2row", [1, D], bf16, kind="ExternalInput").ap()
    wsc_d = nc.dram_tensor("wsc", [1, 2], f32, kind="ExternalInput").ap()
    out_d = nc.dram_tensor("outT", [D, S], f32, kind="ExternalOutput").ap()
    dbg = {}
    if os.environ.get("KERNEL_DEBUG"):
        for nm in ("dbgq", "dbgk", "dbgv", "dbgp", "dbgc"):
            dbg[nm] = nc.dram_tensor(nm, [P, S], bf16, kind="ExternalOutput").ap()

    with tile.TileContext(nc) as tc:
        with ExitStack() as ctx:
            perm = ctx.enter_context(tc.tile_pool(name="perm", bufs=1))
            res_pool = ctx.enter_context(tc.tile_pool(name="res", bufs=7))
            oT_pool = ctx.enter_context(tc.tile_pool(name="oTp", bufs=6))
            bb = ctx.enter_context(tc.tile_pool(name="bigbf", bufs=54))
            fw = ctx.enter_context(tc.tile_pool(name="f32w", bufs=7))
            wpool = ctx.enter_context(tc.tile_pool(name="wpool", bufs=12))

            def bbt(name, shape=(P, S)):
                return bb.tile(list(shape), bf16, name=name, tag="bb")

            def fwt(name, shape=(P, S)):
                return fw.tile(list(shape), f32, name=name, tag="fw")

            def act_recip(out_ap, in_ap):
                # Table-based reciprocal on the (idle) Scalar engine. The
                # nc.scalar.activation wrapper refuses Reciprocal for accuracy
                # reasons; here the inputs are well-conditioned positives
                # (softmax denominators ~1e3, LN std ~1) and table accuracy is
                # far inside this kernel's error budget -- and it takes ~1us
                # vs 6.5us for the exact DVE reciprocal, off the DVE critical
                # path.
                se = nc.scalar
                se.add_instruction(mybir.InstActivation(
                    name=nc.get_next_instruction_name(),
                    func=Act.Reciprocal,
                    ins=[se.lower_ap(in_ap),
                         mybir.ImmediateValue(dtype=f32, value=0.0),
                         mybir.ImmediateValue(dtype=f32, value=1.0),
                         mybir.ImmediateValue(dtype=f32, value=0.0)],
                    outs=[se.lower_ap(out_ap)]))

            ones_kk = perm.tile([P, P], bf16, name="ones_kk")
            nc.gpsimd.memset(ones_kk[:], 1.0)
            ones_row = perm.tile([1, P], bf16, name="ones_row")
            nc.gpsimd.memset(ones_row[:], 1.0)
            eps_t = perm.tile([P, 1], f32, name="eps_t")
            nc.gpsimd.memset(eps_t[:], float(EPS))

            def bias_tile(name, dram, nt):
                t = perm.tile([P, nt], f32, name=name)
                nc.sync.dma_start(t[:], dram.rearrange("(t p) o -> p (t o)", p=P))
                return t

            bq_t = bias_tile("bq_t", bq_d, KT)
            bk_t = bias_tile("bk_t", bk_d, KT)
            bk2_t = bias_tile("bk2_t", bk2_d, KT)
            bo_t = bias_tile("bo_t", bo_d, KT)
            b2_t = bias_tile("b2_t", b2_d, KT)
            b1_t = bias_tile("b1_t", b1_d, FT)
            wsc_t = perm.tile([1, 2], f32, name="wsc_t")
            nc.sync.dma_start(wsc_t[:], wsc_d[:])
            wsc_b = perm.tile([64, 2], f32, name="wsc_b")
            nc.gpsimd.partition_broadcast(wsc_b[:], wsc_t[:])

            # bias rows for natural-layout V, broadcast to 128 partitions
            # via a K=1 ones matmul
            bv_rows = []
            with tc.tile_pool(name="bvp", bufs=1, space="PSUM") as bvp:
                for nm, dram in (("bv", bvrow_d), ("bv2", bv2row_d)):
                    row = perm.tile([1, D], bf16, name=f"{nm}row")
                    nc.sync.dma_start(row[:], dram[:])
                    ps = bvp.tile([P, D], f32, name=f"{nm}ps", tag="bvps")
                    for sl in (slice(0, 512), slice(512, 768)):
                        nc.tensor.matmul(ps[:, sl], ones_row[:], row[:, sl],
                                         start=True, stop=True)
                    bbx = perm.tile([P, D], bf16, name=f"{nm}bcast")
                    nc.vector.tensor_copy(bbx[:], ps[:])
                    bv_rows.append(bbx)
            bv_b, bv2_b = bv_rows

            # ---- load inputs ----
            aT = []
            for kt in range(KT):
                t = res_pool.tile([P, S], f32, name=f"aT{kt}", tag="res")
                nc.sync.dma_start(t[:], aT_d[kt * P:(kt + 1) * P, :])
                aT.append(t)
            oT = []
            for kt in range(KT):
                t = oT_pool.tile([P, S], bf16, name=f"oT{kt}", tag="oT")
                nc.sync.dma_start(t[:], oT_d[kt * P:(kt + 1) * P, :])
                oT.append(t)
            aTb = []
            for kt in range(KT):
                t = bbt(f"aTb{kt}")
                nc.sync.dma_start(t[:], aTb_d[kt * P:(kt + 1) * P, :])
                aTb.append(t)

            # ---- layer norm (transposed layout) ----
            # src: 6 [128,1024] SBUF tiles (f32 or bf16)
            def ln_T(src, src_bf, ln_psum, name):
                tbf, tsq = [], []
                for kt in range(KT):
                    if src_bf is not None:
                        c = src_bf[kt]
                    else:
                        c = bbt(f"{name}bf{kt}")
                        nc.vector.tensor_copy(c[:], src[kt][:])
                    q = bbt(f"{name}sq{kt}")
                    nc.scalar.activation(q[:], c[:], Act.Square)
                    tbf.append(c)
                    tsq.append(q)
                msum = ln_psum.tile([P, S], f32, name=f"{name}ms", tag="lnms")
                sqsum = ln_psum.tile([P, S], f32, name=f"{name}vs", tag="lnvs")
                for kt in range(KT):
                    st, sp = kt == 0, kt == KT - 1
                    for nh in range(2):
                        sl = slice(nh * 512, (nh + 1) * 512)
                        nc.tensor.matmul(msum[:, sl], ones_kk[:], tbf[kt][:, sl],
                                         start=st, stop=sp)
                        nc.tensor.matmul(sqsum[:, sl], ones_kk[:], tsq[kt][:, sl],
                                         start=st, stop=sp)
                m_s = fwt(f"{name}m")
                nc.vector.tensor_scalar(m_s[:], msum[:], 1.0 / D, None, Ax.mult)
                m2 = fwt(f"{name}m2")
                nc.vector.tensor_tensor(m2[:], m_s[:], m_s[:], Ax.mult)
                var = fwt(f"{name}var")
                nc.vector.scalar_tensor_tensor(var[:], sqsum[:], 1.0 / D, m2[:],
                                               Ax.mult, Ax.subtract)
                std = fwt(f"{name}std")
                nc.scalar.activation(std[:], var[:], Act.Sqrt, bias=eps_t[:])
                rstd = fwt(f"{name}rstd")
                act_recip(rstd[:], std[:])
                xn = []
                for kt in range(KT):
                    cen = fwt(f"{name}cen{kt}")
                    nc.vector.scalar_tensor_tensor(cen[:], msum[:], -1.0 / D,
                                                   src[kt][:], Ax.mult, Ax.add)
                    x = bbt(f"{name}xn{kt}")
                    nc.vector.tensor_tensor(x[:], cen[:], rstd[:], Ax.mult)
                    xn.append(x)
                return xn

            with tc.tile_pool(name="lnps_a", bufs=2, space="PSUM") as lnps:
                xnA = ln_T(aT, aTb, lnps, "A")
                xnO = ln_T(oT, oT, lnps, "O")

            # ---- projections ----
            # q is stored per-head zero-padded to full 128 partitions so the
            # score matmul can contract over K=128 (the other head's k rows
            # multiply zero q rows). Keeps the PE array fully lit -> HAM stays
            # at the 2.4 GHz clock.
            qP = [bbt(f"qP{h}") for h in range(NH)]
            kTt = [bbt(f"kT{m}") for m in range(KT)]
            k2T = [bbt(f"k2T{m}") for m in range(KT)]
            # v buffers are 65-strided per head ([v(64) | ones(1)] x 12) with a
            # zeroed tail so the context matmul can take a full 128-wide lhsT
            # slice (rows 65..127 of its PSUM output are ignored).
            vN = [bbt(f"vN{j}") for j in range(JT)]
            v2N = [bbt(f"v2N{j}") for j in range(JT)]

            def load_w(name, kt, cols=None):
                t = wpool.tile([P, D], bf16, name=f"{name}w{kt}", tag="w")
                src = w_d[name]
                if cols is None:
                    nc.sync.dma_start(t[:], src[kt * P:(kt + 1) * P, :])
                else:
                    nc.sync.dma_start(t[:], src[kt * P:(kt + 1) * P,
                                                cols * D:(cols + 1) * D])
                return t

            with tc.tile_pool(name="projps", bufs=3, space="PSUM") as pps:
                # transposed-output projections: q, k, k2
                for pname, xn, bias, dst in (("wq", xnA, bq_t, None),
                                             ("wk", xnA, bk_t, kTt),
                                             ("wk2", xnO, bk2_t, k2T)):
                    wt = [load_w(pname, kt) for kt in range(KT)]
                    for mt in range(KT):
                        ps = pps.tile([P, S], f32, name=f"{pname}ps{mt}", tag="pps")
                        for kt in range(KT):
                            st, sp = kt == 0, kt == KT - 1
                            for nh in range(2):
                                sl = slice(nh * 512, (nh + 1) * 512)
                                nc.tensor.matmul(
                                    ps[:, sl],
                                    wt[kt][:, mt * P:(mt + 1) * P],
                                    xn[kt][:, sl], start=st, stop=sp)
                        if dst is not None:
                            nc.vector.tensor_scalar(dst[mt][:], ps[:],
                                                    bias[:, mt:mt + 1], None,
                                                    Ax.add)
                        else:
                            for hh in range(2):
                                h, po = 2 * mt + hh, hh * 64
                                t = qP[h]
                                nc.gpsimd.memset(t[:], 0.0)
                                nc.vector.tensor_scalar(
                                    t[po:po + 64, :], ps[po:po + 64, :],
                                    bias[po:po + 64, mt:mt + 1], None, Ax.add)
                # natural-layout projections with ones column: v, v2
                for pname, xn, bcast, dst in (("wv", xnA, bv_b, vN),
                                              ("wv2", xnO, bv2_b, v2N)):
                    wt = [load_w(pname, kt) for kt in range(KT)]
                    for jt in range(JT):
                        ps = pps.tile([P, D], f32, name=f"{pname}ps{jt}", tag="pps")
                        for kt in range(KT):
                            st, sp = kt == 0, kt == KT - 1
                            for sl in (slice(0, 512), slice(512, 768)):
                                nc.tensor.matmul(
                                    ps[:, sl],
                                    xn[kt][:, jt * P:(jt + 1) * P],
                                    wt[kt][:, sl], start=st, stop=sp)
                        nc.gpsimd.memset(dst[jt][:], 0.0)
                        dst_v = dst[jt][:, 0:NH * 80].rearrange(
                            "p (h c) -> p h c", c=80)
                        nc.vector.tensor_tensor(
                            dst_v[:, :, 0:64],
                            ps.rearrange("p (h c) -> p h c", c=64)[:],
                            bcast.rearrange("p (h c) -> p h c", c=64)[:],
                            Ax.add)
                        nc.gpsimd.memset(dst_v[:, :, 64:65], 1.0)

            if dbg:
                nc.sync.dma_start(dbg["dbgq"][:], qP[0][:])
                nc.sync.dma_start(dbg["dbgk"][:], kTt[0][:])
                nc.sync.dma_start(dbg["dbgv"][:], vN[0][:])

            # ---- attention ----
            wo_t = [load_w("wo", kt) for kt in range(KT)]
            ctx_t = [bbt(f"ctx{t}") for t in range(KT)]
            with tc.tile_pool(name="attnps", bufs=2, space="PSUM") as aps:
                for h in range(NH):
                    td, po = h // 2, (h % 2) * 64
                    tmps = []
                    for typ, (kk, vv) in enumerate(((kTt, vN), (k2T, v2N))):
                        cacc = aps.tile([P, S], f32, name=f"cv{h}_{typ}", tag="cv")
                        for jt in range(JT):
                            sT = aps.tile([P, S], f32, name=f"sT{h}_{typ}_{jt}",
                                          tag="sT")
                            for nh in range(2):
                                sl = slice(nh * 512, (nh + 1) * 512)
                                nc.tensor.matmul(
                                    sT[:, sl],
                                    kk[td][:, jt * P:(jt + 1) * P],
                                    qP[h][:, sl],
                                    start=True, stop=True)
                            pT = bbt(f"pT{h}_{typ}_{jt}")
                            nc.scalar.activation(pT[:], sT[:], Act.Exp)
                            if dbg and h == 0 and typ == 0 and jt == 0:
                                nc.sync.dma_start(dbg["dbgp"][:], pT[:])
                            for nh in range(2):
                                sl = slice(nh * 512, (nh + 1) * 512)
                                nc.tensor.matmul(
                                    cacc[:, sl],
                                    vv[jt][:, h * 80:h * 80 + P],
                                    pT[:, sl],
                                    start=(jt == 0), stop=(jt == JT - 1))
                        recip = fwt(f"rc{h}_{typ}", (1, S))
                        nc.vector.reciprocal(recip[:], cacc[64:65, :])
                        rb = fwt(f"rb{h}_{typ}", (64, S))
                        nc.gpsimd.partition_broadcast(rb[:], recip[:])
                        tmp = bbt(f"tm{h}_{typ}", (64, S))
                        nc.vector.scalar_tensor_tensor(
                            tmp[:], cacc[0:64, :], wsc_b[0:64, typ:typ + 1],
                            rb[:], Ax.mult, Ax.mult)
                        tmps.append(tmp)
                    nc.vector.tensor_add(ctx_t[td][po:po + 64, :],
                                         tmps[0][:], tmps[1][:])
                    if dbg and h == 1:
                        nc.sync.dma_start(dbg["dbgc"][:], ctx_t[0][:])

            # ---- out-projection + residual ----
            x1 = []
            with tc.tile_pool(name="opps", bufs=2, space="PSUM") as ops:
                for mt in range(KT):
                    ps = ops.tile([P, S], f32, name=f"ops{mt}", tag="ops")
                    for kt in range(KT):
                        st, sp = kt == 0, kt == KT - 1
                        for nh in range(2):
                            sl = slice(nh * 512, (nh + 1) * 512)
                            nc.tensor.matmul(ps[:, sl],
                                             wo_t[kt][:, mt * P:(mt + 1) * P],
                                             ctx_t[kt][:, sl], start=st, stop=sp)
                    t = res_pool.tile([P, S], f32, name=f"x1_{mt}", tag="res")
                    nc.vector.scalar_tensor_tensor(t[:], ps[:], bo_t[:, mt:mt + 1],
                                                   aT[mt][:], Ax.add, Ax.add)
                    x1.append(t)

            # ---- LN2 ----
            with tc.tile_pool(name="lnps_b", bufs=1, space="PSUM") as lnps2:
                xn2 = ln_T(x1, None, lnps2, "B")

            # ---- MLP ----
            with tc.tile_pool(name="mlpps", bufs=3, space="PSUM") as mps:
                hbf = []
                for fq in range(4):
                    w1t = [load_w("w1", kt, cols=fq) for kt in range(KT)]
                    for fl in range(KT):
                        ft = fq * KT + fl
                        ps = mps.tile([P, S], f32, name=f"h_ps{ft}", tag="mps")
                        for kt in range(KT):
                            st, sp = kt == 0, kt == KT - 1
                            for nh in range(2):
                                sl = slice(nh * 512, (nh + 1) * 512)
                                nc.tensor.matmul(ps[:, sl],
                                                 w1t[kt][:, fl * P:(fl + 1) * P],
                                                 xn2[kt][:, sl], start=st, stop=sp)
                        hb = bbt(f"hbf{ft}")
                        nc.scalar.activation(hb[:], ps[:], Act.Gelu_apprx_tanh,
                                             bias=b1_t[:, ft:ft + 1])
                        hbf.append(hb)
                for half in range(2):
                    psl = []
                    for ml in range(3):
                        ps = mps.tile([P, S], f32, name=f"o_ps{half}_{ml}",
                                      tag="mps")
                        psl.append(ps)
                    for kt in range(FT):
                        w2t = load_w("w2", kt)
                        for ml in range(3):
                            mt = half * 3 + ml
                            st, sp = kt == 0, kt == FT - 1
                            for nh in range(2):
                                sl = slice(nh * 512, (nh + 1) * 512)
                                nc.tensor.matmul(psl[ml][:, sl],
                                                 w2t[:, mt * P:(mt + 1) * P],
                                                 hbf[kt][:, sl], start=st, stop=sp)
                    for ml in range(3):
                        mt = half * 3 + ml
                        ot = fwt(f"out{mt}")
                        nc.vector.scalar_tensor_tensor(ot[:], psl[ml][:],
                                                       b2_t[:, mt:mt + 1],
                                                       x1[mt][:], Ax.add, Ax.add)
                        nc.sync.dma_start(out_d[mt * P:(mt + 1) * P, :], ot[:])

    nc.compile()
    return nc


def _get_program():
    global _PROGRAM
    if _PROGRAM is None:
        _PROGRAM = _build_program()
    return _PROGRAM


def _fold_core(inp, b, s):
    """Host-side shard + weight folding for core (batch b, stream s)."""
    if s == 0:
        a, o = inp['x'][b], inp['y'][b]
        g1s, b1s, g1o, b1o = inp['ln1x_g'], inp['ln1x_b'], inp['ln1y_g'], inp['ln1y_b']
        Wq, bq, Wk, bk, Wv, bv = inp['Wq'], inp['bq'], inp['Wk'], inp['bk'], inp['Wv'], inp['bv']
        Wk2, bk2, Wv2, bv2 = inp['Wkd'], inp['bkd'], inp['Wvd'], inp['bvd']
        Wo, bo = inp['Wo'], inp['bo']
        ws, wc = inp['w11'][0], inp['w12'][0]
        g2, b2g = inp['ln2x_g'], inp['ln2x_b']
        W1, b1, W2, b2 = inp['W1'], inp['b1'], inp['W2'], inp['b2']
    else:
        a, o = inp['y'][b], inp['x'][b]
        g1s, b1s, g1o, b1o = inp['ln1y_g'], inp['ln1y_b'], inp['ln1x_g'], inp['ln1x_b']
        Wq, bq, Wk, bk, Wv, bv = inp['Wqd'], inp['bqd'], inp['Wkd'], inp['bkd'], inp['Wvd'], inp['bvd']
        Wk2, bk2, Wv2, bv2 = inp['Wk'], inp['bk'], inp['Wv'], inp['bv']
        Wo, bo = inp['Wod'], inp['bod']
        ws, wc = inp['w21'][0], inp['w22'][0]
        g2, b2g = inp['ln2y_g'], inp['ln2y_b']
        W1, b1, W2, b2 = inp['W1d'], inp['b1d'], inp['W2d'], inp['b2d']

    m = {
        'aT': np.ascontiguousarray(a.T, np.float32),
        'aTb': np.ascontiguousarray(a.T).astype(BF),
        'oT': np.ascontiguousarray(o.T).astype(BF),
        'wq': np.ascontiguousarray(g1s[:, None] * Wq * SCALE).astype(BF),
        'wk': np.ascontiguousarray(g1s[:, None] * Wk).astype(BF),
        'wv': np.ascontiguousarray(g1s[:, None] * Wv).astype(BF),
        'wk2': np.ascontiguousarray(g1o[:, None] * Wk2).astype(BF),
        'wv2': np.ascontiguousarray(g1o[:, None] * Wv2).astype(BF),
        'wo': np.ascontiguousarray(Wo).astype(BF),
        'w1': np.ascontiguousarray(g2[:, None] * W1).astype(BF),
        'w2': np.ascontiguousarray(W2).astype(BF),
        'bq': (SCALE * (bq + b1s @ Wq)).astype(np.float32).reshape(D, 1),
        'bk': (bk + b1s @ Wk).astype(np.float32).reshape(D, 1),
        'bk2': (bk2 + b1o @ Wk2).astype(np.float32).reshape(D, 1),
        'bo': ((ws + wc) * bo).astype(np.float32).reshape(D, 1),
        'b2': np.asarray(b2, np.float32).reshape(D, 1),
        'b1': (b1 + b2g @ W1).astype(np.float32).reshape(F, 1),
        'bvrow': (bv + b1s @ Wv).astype(np.float32).reshape(1, D).astype(BF),
        'bv2row': (bv2 + b1o @ Wv2).astype(np.float32).reshape(1, D).astype(BF),
        'wsc': np.array([[ws, wc]], np.float32),
    }
    return m


LAST_RESULTS = None


def kernel(**inputs):
    from concourse.bass_utils import run_bass_kernel_spmd
    global LAST_RESULTS

    inp = {k: np.asarray(v, np.float32) for k, v in inputs.items()}
    B = inp['x'].shape[0]

    nc = _get_program()
    in_maps = [_fold_core(inp, core // 2, core % 2) for core in range(2 * B)]
    res = run_bass_kernel_spmd(
        nc, in_maps, core_ids=list(range(2 * B)),
        trace=bool(os.environ.get("KERNEL_TRACE")))
    LAST_RESULTS = res

    x_out = np.empty((B, S, D), np.float32)
    y_out = np.empty((B, S, D), np.float32)
    for b in range(B):
        x_out[b] = res.results[2 * b]["outT"].T
        y_out[b] = res.results[2 * b + 1]["outT"].T
    return (x_out, y_out)


# revision 28
# speedup vs baseline: 1.1220x; 1.0120x over previous
"""Trainium2 Bass kernel for the dual-stream transformer block
(nn_Block_87840671138274).

Sharding: 8 cores = 4 batches x 2 streams. Core i handles batch i//2,
stream i%2 (0=x, 1=y) and produces that stream's full output. Each core
redundantly computes the *other* stream's LN + K/V projections (~12%
extra FLOPs) so there are zero collectives.

Layout: everything transposed ([D, S] with D on SBUF partitions).
- Host pre-transposes inputs and pre-folds LN gamma/beta + softmax SCALE
  into the projection weights/biases (exact algebra, f32).
- LN stats (mean / mean-of-squares) via ones-matmul partition reductions.
- Attention computes s^T = k^T.T @ q^T per head (K=64), exp on ACT, and
  the softmax denominator comes from a ones-column appended to V
  (natural layout), accumulated in the same PSUM matmul as the context.
- No max-subtraction in softmax: scores are ~N(0, 0.31), |s| < ~2.
- bf16 matmuls with f32 PSUM accumulation; residual stream kept f32.
"""
import os
import numpy as np
import ml_dtypes

P = 128
S = 1024
D = 768
F = 3072
NH = 12
HD = 64
KT = D // P     # 6
JT = S // P     # 8
FT = F // P     # 24
EPS = 1e-6
SCALE = np.float32(1.0 / np.sqrt(HD))
BF = ml_dtypes.bfloat16

_PROGRAM = None


def _build_program():
    import concourse.bass as bass
    import concourse.bacc as bacc
    import concourse.tile as tile
    from concourse import mybir
    from contextlib import ExitStack

    f32 = mybir.dt.float32
    bf16 = mybir.dt.bfloat16
    Ax = mybir.AluOpType
    Act = mybir.ActivationFunctionType

    nc = bacc.Bacc("TRN2", target_bir_lowering=False, debug=False, num_devices=8)

    aT_d = nc.dram_tensor("aT", [D, S], f32, kind="ExternalInput").ap()
    aTb_d = nc.dram_tensor("aTb", [D, S], bf16, kind="ExternalInput").ap()
    oT_d = nc.dram_tensor("oT", [D, S], bf16, kind="ExternalInput").ap()
    w_d = {}
    for w in ("wq", "wk", "wv", "wk2", "wv2", "wo"):
        w_d[w] = nc.dram_tensor(w, [D, D], bf16, kind="ExternalInput").ap()
    w_d["w1"] = nc.dram_tensor("w1", [D, F], bf16, kind="ExternalInput").ap()
    w_d["w2"] = nc.dram_tensor("w2", [F, D], bf16, kind="ExternalInput").ap()
    bq_d = nc.dram_tensor("bq", [D, 1], f32, kind="ExternalInput").ap()
    bk_d = nc.dram_tensor("bk", [D, 1], f32, kind="ExternalInput").ap()
    bk2_d = nc.dram_tensor("bk2", [D, 1], f32, kind="ExternalInput").ap()
    bo_d = nc.dram_tensor("bo", [D, 1], f32, kind="ExternalInput").ap()
    b2_d = nc.dram_tensor("b2", [D, 1], f32, kind="ExternalInput").ap()
    b1_d = nc.dram_tensor("b1", [F, 1], f32, kind="ExternalInput").ap()
    bvrow_d = nc.dram_tensor("bvrow", [1, D], bf16, kind="ExternalInput").ap()
    bv2row_d = nc.dram_tensor("bv2row", [1, D], bf16, kind="ExternalInput").ap()
    wsc_d = nc.dram_tensor("wsc", [1, 2], f32, kind="ExternalInput").ap()
    out_d = nc.dram_tensor("outT", [D, S], f32, kind="ExternalOutput").ap()
    dbg = {}
    if os.environ.get("KERNEL_DEBUG"):
        for nm in ("dbgq", "dbgk", "dbgv", "dbgp", "dbgc"):
            dbg[nm] = nc.dram_tensor(nm, [P, S], bf16, kind="ExternalOutput").ap()

    with tile.TileContext(nc) as tc:
        with ExitStack() as ctx:
            perm = ctx.enter_context(tc.tile_pool(name="perm", bufs=1))
            res_pool = ctx.enter_context(tc.tile_pool(name="res", bufs=7))
            oT_pool = ctx.enter_context(tc.tile_pool(name="oTp", bufs=6))
            bb = ctx.enter_context(tc.tile_pool(name="bigbf", bufs=54))
            fw = ctx.enter_context(tc.tile_pool(name="f32w", bufs=7))
            wpool = ctx.enter_context(tc.tile_pool(name="wpool", bufs=12))

            def bbt(name, shape=(P, S)):
                return bb.tile(list(shape), bf16, name=name, tag="bb")

            def fwt(name, shape=(P, S)):
                return fw.tile(list(shape), f32, name=name, tag="fw")

            def act_recip(out_ap, in_ap):
                # Table-based reciprocal on the (idle) Scalar engine. The
                # nc.scalar.activation wrapper refuses Reciprocal for accuracy
                # reasons; here the inputs are well-conditioned positives
                # (softmax denominators ~1e3, LN std ~1) and table accuracy is
                # far inside this kernel's error budget -- and it takes ~1us
                # vs 6.5us for the exact DVE reciprocal, off the DVE critical
                # path.
                se = nc.scalar
                se.add_instruction(mybir.InstActivation(
                    name=nc.get_next_instruction_name(),
                    func=Act.Reciprocal,
                    ins=[se.lower_ap(in_ap),
                         mybir.ImmediateValue(dtype=f32, value=0.0),
                         mybir.ImmediateValue(dtype=f32, value=1.0),
                         mybir.ImmediateValue(dtype=f32, value=0.0)],
                    outs=[se.lower_ap(out_ap)]))

            ones_kk = perm.tile([P, P], bf16, name="ones_kk")
            nc.gpsimd.memset(ones_kk[:], 1.0)
            ones_row = perm.tile([1, P], bf16, name="ones_row")
            nc.gpsimd.memset(ones_row[:], 1.0)
            eps_t = perm.tile([P, 1], f32, name="eps_t")
            nc.gpsimd.memset(eps_t[:], float(EPS))

            aTb = []
            for kt in range(KT):
                t = bbt(f"aTb{kt}")
                nc.sync.dma_start(t[:], aTb_d[kt * P:(kt + 1) * P, :])
                aTb.append(t)
            oT = []
            for kt in range(KT):
                t = oT_pool.tile([P, S], bf16, name=f"oT{kt}", tag="oT")
                nc.sync.dma_start(t[:], oT_d[kt * P:(kt + 1) * P, :])
                oT.append(t)

            def bias_tile(name, dram, nt):
                t = perm.tile([P, nt], f32, name=name)
                nc.sync.dma_start(t[:], dram.rearrange("(t p) o -> p (t o)", p=P))
                return t

            bq_t = bias_tile("bq_t", bq_d, KT)
            bk_t = bias_tile("bk_t", bk_d, KT)
            bk2_t = bias_tile("bk2_t", bk2_d, KT)
            bo_t = bias_tile("bo_t", bo_d, KT)
            b2_t = bias_tile("b2_t", b2_d, KT)
            b1_t = bias_tile("b1_t", b1_d, FT)
            wsc_t = perm.tile([1, 2], f32, name="wsc_t")
            nc.sync.dma_start(wsc_t[:], wsc_d[:])
            wsc_b = perm.tile([64, 2], f32, name="wsc_b")
            nc.gpsimd.partition_broadcast(wsc_b[:], wsc_t[:])

            # bias rows for natural-layout V, broadcast to 128 partitions
            # via a K=1 ones matmul
            bv_rows = []
            with tc.tile_pool(name="bvp", bufs=1, space="PSUM") as bvp:
                for nm, dram in (("bv", bvrow_d), ("bv2", bv2row_d)):
                    row = perm.tile([1, D], bf16, name=f"{nm}row")
                    nc.sync.dma_start(row[:], dram[:])
                    ps = bvp.tile([P, D], f32, name=f"{nm}ps", tag="bvps")
                    for sl in (slice(0, 512), slice(512, 768)):
                        nc.tensor.matmul(ps[:, sl], ones_row[:], row[:, sl],
                                         start=True, stop=True)
                    bbx = perm.tile([P, D], bf16, name=f"{nm}bcast")
                    nc.vector.tensor_copy(bbx[:], ps[:])
                    bv_rows.append(bbx)
            bv_b, bv2_b = bv_rows

            # ---- load inputs ----
            aT = []
            for kt in range(KT):
                t = res_pool.tile([P, S], f32, name=f"aT{kt}", tag="res")
                nc.sync.dma_start(t[:], aT_d[kt * P:(kt + 1) * P, :])
                aT.append(t)


            # ---- layer norm (transposed layout) ----
            # src: 6 [128,1024] SBUF tiles (f32 or bf16)
            def ln_T(src, src_bf, ln_psum, name):
                tbf, tsq = [], []
                for kt in range(KT):
                    if src_bf is not None:
                        c = src_bf[kt]
                    else:
                        c = bbt(f"{name}bf{kt}")
                        nc.vector.tensor_copy(c[:], src[kt][:])
                    q = bbt(f"{name}sq{kt}")
                    nc.scalar.activation(q[:], c[:], Act.Square)
                    tbf.append(c)
                    tsq.append(q)
                msum = ln_psum.tile([P, S], f32, name=f"{name}ms", tag="lnms")
                sqsum = ln_psum.tile([P, S], f32, name=f"{name}vs", tag="lnvs")
                for kt in range(KT):
                    st, sp = kt == 0, kt == KT - 1
                    for nh in range(2):
                        sl = slice(nh * 512, (nh + 1) * 512)
                        nc.tensor.matmul(msum[:, sl], ones_kk[:], tbf[kt][:, sl],
                                         start=st, stop=sp)
                        nc.tensor.matmul(sqsum[:, sl], ones_kk[:], tsq[kt][:, sl],
                                         start=st, stop=sp)
                m_s = fwt(f"{name}m")
                nc.vector.tensor_scalar(m_s[:], msum[:], 1.0 / D, None, Ax.mult)
                m2 = fwt(f"{name}m2")
                nc.vector.tensor_tensor(m2[:], m_s[:], m_s[:], Ax.mult)
                var = fwt(f"{name}var")
                nc.vector.scalar_tensor_tensor(var[:], sqsum[:], 1.0 / D, m2[:],
                                               Ax.mult, Ax.subtract)
                std = fwt(f"{name}std")
                nc.scalar.activation(std[:], var[:], Act.Sqrt, bias=eps_t[:])
                rstd = fwt(f"{name}rstd")
                act_recip(rstd[:], std[:])
                xn = []
                for kt in range(KT):
                    cen = fwt(f"{name}cen{kt}")
                    nc.vector.scalar_tensor_tensor(cen[:], msum[:], -1.0 / D,
                                                   src[kt][:], Ax.mult, Ax.add)
                    x = bbt(f"{name}xn{kt}")
                    nc.vector.tensor_tensor(x[:], cen[:], rstd[:], Ax.mult)
                    xn.append(x)
                return xn

            with tc.tile_pool(name="lnps_a", bufs=2, space="PSUM") as lnps:
                xnA = ln_T(aT, aTb, lnps, "A")
                xnO = ln_T(oT, oT, lnps, "O")

            # ---- projections ----
            # q is stored per-head zero-padded to full 128 partitions so the
            # score matmul can contract over K=128 (the other head's k rows
            # multiply zero q rows). Keeps the PE array fully lit -> HAM stays
            # at the 2.4 GHz clock.
            qP = [bbt(f"qP{h}") for h in range(NH)]
            kTt = [bbt(f"kT{m}") for m in range(KT)]
            k2T = [bbt(f"k2T{m}") for m in range(KT)]
            # v buffers are 65-strided per head ([v(64) | ones(1)] x 12) with a
            # zeroed tail so the context matmul can take a full 128-wide lhsT
            # slice (rows 65..127 of its PSUM output are ignored).
            vN = [bbt(f"vN{j}") for j in range(JT)]
            v2N = [bbt(f"v2N{j}") for j in range(JT)]

            def load_w(name, kt, cols=None):
                t = wpool.tile([P, D], bf16, name=f"{name}w{kt}", tag="w")
                src = w_d[name]
                if cols is None:
                    nc.sync.dma_start(t[:], src[kt * P:(kt + 1) * P, :])
                else:
                    nc.sync.dma_start(t[:], src[kt * P:(kt + 1) * P,
                                                cols * D:(cols + 1) * D])
                return t

            with tc.tile_pool(name="projps", bufs=3, space="PSUM") as pps:
                # transposed-output projections: q, k, k2
                for pname, xn, bias, dst in (("wq", xnA, bq_t, None),
                                             ("wk", xnA, bk_t, kTt),
                                             ("wk2", xnO, bk2_t, k2T)):
                    wt = [load_w(pname, kt) for kt in range(KT)]
                    for mt in range(KT):
                        ps = pps.tile([P, S], f32, name=f"{pname}ps{mt}", tag="pps")
                        for kt in range(KT):
                            st, sp = kt == 0, kt == KT - 1
                            for nh in range(2):
                                sl = slice(nh * 512, (nh + 1) * 512)
                                nc.tensor.matmul(
                                    ps[:, sl],
                                    wt[kt][:, mt * P:(mt + 1) * P],
                                    xn[kt][:, sl], start=st, stop=sp)
                        if dst is not None:
                            nc.vector.tensor_scalar(dst[mt][:], ps[:],
                                                    bias[:, mt:mt + 1], None,
                                                    Ax.add)
                        else:
                            for hh in range(2):
                                h, po = 2 * mt + hh, hh * 64
                                t = qP[h]
                                nc.gpsimd.memset(t[:], 0.0)
                                nc.vector.tensor_scalar(
                                    t[po:po + 64, :], ps[po:po + 64, :],
                                    bias[po:po + 64, mt:mt + 1], None, Ax.add)
                # natural-layout projections with ones column: v, v2
                for pname, xn, bcast, dst in (("wv", xnA, bv_b, vN),
                                              ("wv2", xnO, bv2_b, v2N)):
                    wt = [load_w(pname, kt) for kt in range(KT)]
                    for jt in range(JT):
                        ps = pps.tile([P, D], f32, name=f"{pname}ps{jt}", tag="pps")
                        for kt in range(KT):
                            st, sp = kt == 0, kt == KT - 1
                            for sl in (slice(0, 512), slice(512, 768)):
                                nc.tensor.matmul(
                                    ps[:, sl],
                                    xn[kt][:, jt * P:(jt + 1) * P],
                                    wt[kt][:, sl], start=st, stop=sp)
                        nc.gpsimd.memset(dst[jt][:], 0.0)
                        dst_v = dst[jt][:, 0:NH * 80].rearrange(
                            "p (h c) -> p h c", c=80)
                        nc.vector.tensor_tensor(
                            dst_v[:, :, 0:64],
                            ps.rearrange("p (h c) -> p h c", c=64)[:],
                            bcast.rearrange("p (h c) -> p h c", c=64)[:],
                            Ax.add)
                        nc.gpsimd.memset(dst_v[:, :, 64:65], 1.0)

            if dbg:
                nc.sync.dma_start(dbg["dbgq"][:], qP[0][:])
                nc.sync.dma_start(dbg["dbgk"][:], kTt[0][:])
                nc.sync.dma_start(dbg["dbgv"][:], vN[0][:])

            # ---- attention ----
            wo_t = [load_w("wo", kt) for kt in range(KT)]
            ctx_t = [bbt(f"ctx{t}") for t in range(KT)]
            with tc.tile_pool(name="attnps", bufs=2, space="PSUM") as aps:
                for h in range(NH):
                    td, po = h // 2, (h % 2) * 64
                    tmps = []
                    for typ, (kk, vv) in enumerate(((kTt, vN), (k2T, v2N))):
                        cacc = aps.tile([P, S], f32, name=f"cv{h}_{typ}", tag="cv")
                        for jt in range(JT):
                            sT = aps.tile([P, S], f32, name=f"sT{h}_{typ}_{jt}",
                                          tag="sT")
                            for nh in range(2):
                                sl = slice(nh * 512, (nh + 1) * 512)
                                nc.tensor.matmul(
                                    sT[:, sl],
                                    kk[td][:, jt * P:(jt + 1) * P],
                                    qP[h][:, sl],
                                    start=True, stop=True)
                            pT = bbt(f"pT{h}_{typ}_{jt}")
                            nc.scalar.activation(pT[:], sT[:], Act.Exp)
                            if dbg and h == 0 and typ == 0 and jt == 0:
                                nc.sync.dma_start(dbg["dbgp"][:], pT[:])
                            for nh in range(2):
                                sl = slice(nh * 512, (nh + 1) * 512)
                                nc.tensor.matmul(
                                    cacc[:, sl],
                                    vv[jt][:, h * 80:h * 80 + P],
                                    pT[:, sl],
                                    start=(jt == 0), stop=(jt == JT - 1))
                        recip = fwt(f"rc{h}_{typ}", (1, S))
                        nc.vector.reciprocal(recip[:], cacc[64:65, :])
                        rb = fwt(f"rb{h}_{typ}", (64, S))
                        nc.gpsimd.partition_broadcast(rb[:], recip[:])
                        tmp = bbt(f"tm{h}_{typ}", (64, S))
                        nc.vector.scalar_tensor_tensor(
                            tmp[:], cacc[0:64, :], wsc_b[0:64, typ:typ + 1],
                            rb[:], Ax.mult, Ax.mult)
                        tmps.append(tmp)
                    nc.vector.tensor_add(ctx_t[td][po:po + 64, :],
                                         tmps[0][:], tmps[1][:])
                    if dbg and h == 1:
                        nc.sync.dma_start(dbg["dbgc"][:], ctx_t[0][:])

            # ---- out-projection + residual ----
            x1 = []
            with tc.tile_pool(name="opps", bufs=2, space="PSUM") as ops:
                for mt in range(KT):
                    ps = ops.tile([P, S], f32, name=f"ops{mt}", tag="ops")
                    for kt in range(KT):
                        st, sp = kt == 0, kt == KT - 1
                        for nh in range(2):
                            sl = slice(nh * 512, (nh + 1) * 512)
                            nc.tensor.matmul(ps[:, sl],
                                             wo_t[kt][:, mt * P:(mt + 1) * P],
                                             ctx_t[kt][:, sl], start=st, stop=sp)
                    t = res_pool.tile([P, S], f32, name=f"x1_{mt}", tag="res")
                    nc.vector.scalar_tensor_tensor(t[:], ps[:], bo_t[:, mt:mt + 1],
                                                   aT[mt][:], Ax.add, Ax.add)
                    x1.append(t)

            # ---- LN2 ----
            with tc.tile_pool(name="lnps_b", bufs=1, space="PSUM") as lnps2:
                xn2 = ln_T(x1, None, lnps2, "B")

            # ---- MLP ----
            with tc.tile_pool(name="mlpps", bufs=3, space="PSUM") as mps:
                hbf = []
                for fq in range(4):
                    w1t = [load_w("w1", kt, cols=fq) for kt in range(KT)]
                    for fl in range(KT):
                        ft = fq * KT + fl
                        ps = mps.tile([P, S], f32, name=f"h_ps{ft}", tag="mps")
                        for kt in range(KT):
                            st, sp = kt == 0, kt == KT - 1
                            for nh in range(2):
                                sl = slice(nh * 512, (nh + 1) * 512)
                                nc.tensor.matmul(ps[:, sl],
                                                 w1t[kt][:, fl * P:(fl + 1) * P],
                                                 xn2[kt][:, sl], start=st, stop=sp)
                        hb = bbt(f"hbf{ft}")
                        nc.scalar.activation(hb[:], ps[:], Act.Gelu_apprx_tanh,
                                             bias=b1_t[:, ft:ft + 1])
                        hbf.append(hb)
                for half in range(2):
                    psl = []
                    for ml in range(3):
                        ps = mps.tile([P, S], f32, name=f"o_ps{half}_{ml}",
                                      tag="mps")
                        psl.append(ps)
                    for kt in range(FT):
                        w2t = load_w("w2", kt)
                        for ml in range(3):
                            mt = half * 3 + ml
                            st, sp = kt == 0, kt == FT - 1
                            for nh in range(2):
                                sl = slice(nh * 512, (nh + 1) * 512)
                                nc.tensor.matmul(psl[ml][:, sl],
                                                 w2t[:, mt * P:(mt + 1) * P],
                                                 hbf[kt][:, sl], start=st, stop=sp)
                    for ml in range(3):
                        mt = half * 3 + ml
                        ot = fwt(f"out{mt}")
                        nc.vector.scalar_tensor_tensor(ot[:], psl[ml][:],
                                                       b2_t[:, mt:mt + 1],
                                                       x1[mt][:], Ax.add, Ax.add)
                        nc.sync.dma_start(out_d[mt * P:(mt + 1) * P, :], ot[:])

    nc.compile()
    return nc


def _get_program():
    global _PROGRAM
    if _PROGRAM is None:
        _PROGRAM = _build_program()
    return _PROGRAM


def _fold_core(inp, b, s):
    """Host-side shard + weight folding for core (batch b, stream s)."""
    if s == 0:
        a, o = inp['x'][b], inp['y'][b]
        g1s, b1s, g1o, b1o = inp['ln1x_g'], inp['ln1x_b'], inp['ln1y_g'], inp['ln1y_b']
        Wq, bq, Wk, bk, Wv, bv = inp['Wq'], inp['bq'], inp['Wk'], inp['bk'], inp['Wv'], inp['bv']
        Wk2, bk2, Wv2, bv2 = inp['Wkd'], inp['bkd'], inp['Wvd'], inp['bvd']
        Wo, bo = inp['Wo'], inp['bo']
        ws, wc = inp['w11'][0], inp['w12'][0]
        g2, b2g = inp['ln2x_g'], inp['ln2x_b']
        W1, b1, W2, b2 = inp['W1'], inp['b1'], inp['W2'], inp['b2']
    else:
        a, o = inp['y'][b], inp['x'][b]
        g1s, b1s, g1o, b1o = inp['ln1y_g'], inp['ln1y_b'], inp['ln1x_g'], inp['ln1x_b']
        Wq, bq, Wk, bk, Wv, bv = inp['Wqd'], inp['bqd'], inp['Wkd'], inp['bkd'], inp['Wvd'], inp['bvd']
        Wk2, bk2, Wv2, bv2 = inp['Wk'], inp['bk'], inp['Wv'], inp['bv']
        Wo, bo = inp['Wod'], inp['bod']
        ws, wc = inp['w21'][0], inp['w22'][0]
        g2, b2g = inp['ln2y_g'], inp['ln2y_b']
        W1, b1, W2, b2 = inp['W1d'], inp['b1d'], inp['W2d'], inp['b2d']

    m = {
        'aT': np.ascontiguousarray(a.T, np.float32),
        'aTb': np.ascontiguousarray(a.T).astype(BF),
        'oT': np.ascontiguousarray(o.T).astype(BF),
        'wq': np.ascontiguousarray(g1s[:, None] * Wq * SCALE).astype(BF),
        'wk': np.ascontiguousarray(g1s[:, None] * Wk).astype(BF),
        'wv': np.ascontiguousarray(g1s[:, None] * Wv).astype(BF),
        'wk2': np.ascontiguousarray(g1o[:, None] * Wk2).astype(BF),
        'wv2': np.ascontiguousarray(g1o[:, None] * Wv2).astype(BF),
        'wo': np.ascontiguousarray(Wo).astype(BF),
        'w1': np.ascontiguousarray(g2[:, None] * W1).astype(BF),
        'w2': np.ascontiguousarray(W2).astype(BF),
        'bq': (SCALE * (bq + b1s @ Wq)).astype(np.float32).reshape(D, 1),
        'bk': (bk + b1s @ Wk).astype(np.float32).reshape(D, 1),
        'bk2': (bk2 + b1o @ Wk2).astype(np.float32).reshape(D, 1),
        'bo': ((ws + wc) * bo).astype(np.float32).reshape(D, 1),
        'b2': np.asarray(b2, np.float32).reshape(D, 1),
        'b1': (b1 + b2g @ W1).astype(np.float32).reshape(F, 1),
        'bvrow': (bv + b1s @ Wv).astype(np.float32).reshape(1, D).astype(BF),
        'bv2row': (bv2 + b1o @ Wv2).astype(np.float32).reshape(1, D).astype(BF),
        'wsc': np.array([[ws, wc]], np.float32),
    }
    return m


LAST_RESULTS = None


def kernel(**inputs):
    from concourse.bass_utils import run_bass_kernel_spmd
    global LAST_RESULTS

    inp = {k: np.asarray(v, np.float32) for k, v in inputs.items()}
    B = inp['x'].shape[0]

    nc = _get_program()
    in_maps = [_fold_core(inp, core // 2, core % 2) for core in range(2 * B)]
    res = run_bass_kernel_spmd(
        nc, in_maps, core_ids=list(range(2 * B)),
        trace=bool(os.environ.get("KERNEL_TRACE")))
    LAST_RESULTS = res

    x_out = np.empty((B, S, D), np.float32)
    y_out = np.empty((B, S, D), np.float32)
    for b in range(B):
        x_out[b] = res.results[2 * b]["outT"].T
        y_out[b] = res.results[2 * b + 1]["outT"].T
    return (x_out, y_out)


# revision 29
# speedup vs baseline: 1.1274x; 1.0048x over previous
"""Trainium2 Bass kernel for the dual-stream transformer block
(nn_Block_87840671138274).

Sharding: 8 cores = 4 batches x 2 streams. Core i handles batch i//2,
stream i%2 (0=x, 1=y) and produces that stream's full output. Each core
redundantly computes the *other* stream's LN + K/V projections (~12%
extra FLOPs) so there are zero collectives.

Layout: everything transposed ([D, S] with D on SBUF partitions).
- Host pre-transposes inputs and pre-folds LN gamma/beta + softmax SCALE
  into the projection weights/biases (exact algebra, f32).
- LN stats (mean / mean-of-squares) via ones-matmul partition reductions.
- Attention computes s^T = k^T.T @ q^T per head, exp on ACT, and the
  softmax denominator comes from a ones-column appended to V (natural
  layout), accumulated in the same PSUM matmul as the context.
- All matmuls are full 128x128 (q zero-padded per head, v lhsT widened
  into the neighbor's zero gap) -- partial-array matmuls (K=64 / M=65)
  made the PE HAM clock-gate hold the array at 1.2 GHz for the whole
  attention phase, doubling their duration.
- No max-subtraction in softmax: scores are ~N(0, 0.31), |s| < ~2.
- bf16 matmuls with f32 PSUM accumulation; residual stream kept f32.
- Reciprocals: LN rstd on the Scalar engine (table Reciprocal, ~1us,
  off the DVE critical path); softmax denominators stay on the exact
  DVE reciprocal (a Scalar-engine reciprocal there thrashes the ACT
  table against EXP, +90us).
"""
import os
import numpy as np
import ml_dtypes

P = 128
S = 1024
D = 768
F = 3072
NH = 12
HD = 64
KT = D // P     # 6
JT = S // P     # 8
FT = F // P     # 24
EPS = 1e-6
SCALE = np.float32(1.0 / np.sqrt(HD))
BF = ml_dtypes.bfloat16

_PROGRAM = None


def _build_program():
    import concourse.bass as bass
    import concourse.bacc as bacc
    import concourse.tile as tile
    from concourse import mybir
    from contextlib import ExitStack

    f32 = mybir.dt.float32
    bf16 = mybir.dt.bfloat16
    Ax = mybir.AluOpType
    Act = mybir.ActivationFunctionType

    nc = bacc.Bacc("TRN2", target_bir_lowering=False, debug=False, num_devices=8)

    aT_d = nc.dram_tensor("aT", [D, S], f32, kind="ExternalInput").ap()
    aTb_d = nc.dram_tensor("aTb", [D, S], bf16, kind="ExternalInput").ap()
    oT_d = nc.dram_tensor("oT", [D, S], bf16, kind="ExternalInput").ap()
    w_d = {}
    for w in ("wq", "wk", "wv", "wk2", "wv2", "wo"):
        w_d[w] = nc.dram_tensor(w, [D, D], bf16, kind="ExternalInput").ap()
    w_d["w1"] = nc.dram_tensor("w1", [D, F], bf16, kind="ExternalInput").ap()
    w_d["w2"] = nc.dram_tensor("w2", [F, D], bf16, kind="ExternalInput").ap()
    bq_d = nc.dram_tensor("bq", [D, 1], f32, kind="ExternalInput").ap()
    bk_d = nc.dram_tensor("bk", [D, 1], f32, kind="ExternalInput").ap()
    bk2_d = nc.dram_tensor("bk2", [D, 1], f32, kind="ExternalInput").ap()
    bo_d = nc.dram_tensor("bo", [D, 1], f32, kind="ExternalInput").ap()
    b2_d = nc.dram_tensor("b2", [D, 1], f32, kind="ExternalInput").ap()
    b1_d = nc.dram_tensor("b1", [F, 1], f32, kind="ExternalInput").ap()
    bvrow_d = nc.dram_tensor("bvrow", [1, D], bf16, kind="ExternalInput").ap()
    bv2row_d = nc.dram_tensor("bv2row", [1, D], bf16, kind="ExternalInput").ap()
    wsc_d = nc.dram_tensor("wsc", [1, 2], f32, kind="ExternalInput").ap()
    out_d = nc.dram_tensor("outT", [D, S], f32, kind="ExternalOutput").ap()
    dbg = {}
    if os.environ.get("KERNEL_DEBUG"):
        for nm in ("dbgq", "dbgk", "dbgv", "dbgp", "dbgc"):
            dbg[nm] = nc.dram_tensor(nm, [P, S], bf16, kind="ExternalOutput").ap()

    with tile.TileContext(nc) as tc:
        with ExitStack() as ctx:
            perm = ctx.enter_context(tc.tile_pool(name="perm", bufs=1))
            res_pool = ctx.enter_context(tc.tile_pool(name="res", bufs=7))
            oT_pool = ctx.enter_context(tc.tile_pool(name="oTp", bufs=6))
            bb = ctx.enter_context(tc.tile_pool(name="bigbf", bufs=54))
            fw = ctx.enter_context(tc.tile_pool(name="f32w", bufs=7))
            wpool = ctx.enter_context(tc.tile_pool(name="wpool", bufs=12))

            def bbt(name, shape=(P, S)):
                return bb.tile(list(shape), bf16, name=name, tag="bb")

            def fwt(name, shape=(P, S)):
                return fw.tile(list(shape), f32, name=name, tag="fw")

            def act_recip(out_ap, in_ap):
                # Table-based reciprocal on the (idle) Scalar engine. The
                # nc.scalar.activation wrapper refuses Reciprocal for accuracy
                # reasons; here the inputs are well-conditioned positives
                # (softmax denominators ~1e3, LN std ~1) and table accuracy is
                # far inside this kernel's error budget -- and it takes ~1us
                # vs 6.5us for the exact DVE reciprocal, off the DVE critical
                # path.
                se = nc.scalar
                se.add_instruction(mybir.InstActivation(
                    name=nc.get_next_instruction_name(),
                    func=Act.Reciprocal,
                    ins=[se.lower_ap(in_ap),
                         mybir.ImmediateValue(dtype=f32, value=0.0),
                         mybir.ImmediateValue(dtype=f32, value=1.0),
                         mybir.ImmediateValue(dtype=f32, value=0.0)],
                    outs=[se.lower_ap(out_ap)]))

            ones_kk = perm.tile([P, P], bf16, name="ones_kk")
            nc.gpsimd.memset(ones_kk[:], 1.0)
            ones_row = perm.tile([1, P], bf16, name="ones_row")
            nc.gpsimd.memset(ones_row[:], 1.0)
            eps_t = perm.tile([P, 1], f32, name="eps_t")
            nc.gpsimd.memset(eps_t[:], float(EPS))

            aTb = []
            for kt in range(KT):
                t = bbt(f"aTb{kt}")
                nc.sync.dma_start(t[:], aTb_d[kt * P:(kt + 1) * P, :])
                aTb.append(t)
            oT = []
            for kt in range(KT):
                t = oT_pool.tile([P, S], bf16, name=f"oT{kt}", tag="oT")
                nc.sync.dma_start(t[:], oT_d[kt * P:(kt + 1) * P, :])
                oT.append(t)

            def bias_tile(name, dram, nt):
                t = perm.tile([P, nt], f32, name=name)
                nc.sync.dma_start(t[:], dram.rearrange("(t p) o -> p (t o)", p=P))
                return t

            bq_t = bias_tile("bq_t", bq_d, KT)
            bk_t = bias_tile("bk_t", bk_d, KT)
            bk2_t = bias_tile("bk2_t", bk2_d, KT)
            bo_t = bias_tile("bo_t", bo_d, KT)
            b2_t = bias_tile("b2_t", b2_d, KT)
            b1_t = bias_tile("b1_t", b1_d, FT)
            wsc_t = perm.tile([1, 2], f32, name="wsc_t")
            nc.sync.dma_start(wsc_t[:], wsc_d[:])
            wsc_b = perm.tile([64, 2], f32, name="wsc_b")
            nc.gpsimd.partition_broadcast(wsc_b[:], wsc_t[:])

            # bias rows for natural-layout V, broadcast to 128 partitions
            # via a K=1 ones matmul
            bv_rows = []
            with tc.tile_pool(name="bvp", bufs=1, space="PSUM") as bvp:
                for nm, dram in (("bv", bvrow_d), ("bv2", bv2row_d)):
                    row = perm.tile([1, D], bf16, name=f"{nm}row")
                    nc.sync.dma_start(row[:], dram[:])
                    ps = bvp.tile([P, D], f32, name=f"{nm}ps", tag="bvps")
                    for sl in (slice(0, 512), slice(512, 768)):
                        nc.tensor.matmul(ps[:, sl], ones_row[:], row[:, sl],
                                         start=True, stop=True)
                    bbx = perm.tile([P, D], bf16, name=f"{nm}bcast")
                    nc.vector.tensor_copy(bbx[:], ps[:])
                    bv_rows.append(bbx)
            bv_b, bv2_b = bv_rows

            # ---- load inputs ----
            aT = []
            for kt in range(KT):
                t = res_pool.tile([P, S], f32, name=f"aT{kt}", tag="res")
                nc.sync.dma_start(t[:], aT_d[kt * P:(kt + 1) * P, :])
                aT.append(t)


            # ---- layer norm (transposed layout) ----
            # src: 6 [128,1024] SBUF tiles (f32 or bf16)
            def ln_T(src, src_bf, ln_psum, name):
                tbf, tsq = [], []
                for kt in range(KT):
                    if src_bf is not None:
                        c = src_bf[kt]
                    else:
                        c = bbt(f"{name}bf{kt}")
                        nc.vector.tensor_copy(c[:], src[kt][:])
                    q = bbt(f"{name}sq{kt}")
                    nc.scalar.activation(q[:], c[:], Act.Square)
                    tbf.append(c)
                    tsq.append(q)
                msum = ln_psum.tile([P, S], f32, name=f"{name}ms", tag="lnms")
                sqsum = ln_psum.tile([P, S], f32, name=f"{name}vs", tag="lnvs")
                for kt in range(KT):
                    st, sp = kt == 0, kt == KT - 1
                    for nh in range(2):
                        sl = slice(nh * 512, (nh + 1) * 512)
                        nc.tensor.matmul(msum[:, sl], ones_kk[:], tbf[kt][:, sl],
                                         start=st, stop=sp)
                        nc.tensor.matmul(sqsum[:, sl], ones_kk[:], tsq[kt][:, sl],
                                         start=st, stop=sp)
                m_s = fwt(f"{name}m")
                nc.vector.tensor_scalar(m_s[:], msum[:], 1.0 / D, None, Ax.mult)
                m2 = fwt(f"{name}m2")
                nc.vector.tensor_tensor(m2[:], m_s[:], m_s[:], Ax.mult)
                var = fwt(f"{name}var")
                nc.vector.scalar_tensor_tensor(var[:], sqsum[:], 1.0 / D, m2[:],
                                               Ax.mult, Ax.subtract)
                std = fwt(f"{name}std")
                nc.scalar.activation(std[:], var[:], Act.Sqrt, bias=eps_t[:])
                rstd = fwt(f"{name}rstd")
                act_recip(rstd[:], std[:])
                xn = []
                for kt in range(KT):
                    cen = fwt(f"{name}cen{kt}")
                    nc.vector.scalar_tensor_tensor(cen[:], msum[:], -1.0 / D,
                                                   src[kt][:], Ax.mult, Ax.add)
                    x = bbt(f"{name}xn{kt}")
                    nc.vector.tensor_tensor(x[:], cen[:], rstd[:], Ax.mult)
                    xn.append(x)
                return xn

            with tc.tile_pool(name="lnps_a", bufs=2, space="PSUM") as lnps:
                xnA = ln_T(aT, aTb, lnps, "A")
                xnO = ln_T(oT, oT, lnps, "O")

            # ---- projections ----
            # q is stored per-head zero-padded to full 128 partitions so the
            # score matmul can contract over K=128 (the other head's k rows
            # multiply zero q rows). Keeps the PE array fully lit -> HAM stays
            # at the 2.4 GHz clock.
            qP = [bbt(f"qP{h}") for h in range(NH)]
            kTt = [bbt(f"kT{m}") for m in range(KT)]
            k2T = [bbt(f"k2T{m}") for m in range(KT)]
            # v buffers are 65-strided per head ([v(64) | ones(1)] x 12) with a
            # zeroed tail so the context matmul can take a full 128-wide lhsT
            # slice (rows 65..127 of its PSUM output are ignored).
            vN = [bbt(f"vN{j}") for j in range(JT)]
            v2N = [bbt(f"v2N{j}") for j in range(JT)]

            def load_w(name, kt, cols=None):
                t = wpool.tile([P, D], bf16, name=f"{name}w{kt}", tag="w")
                src = w_d[name]
                if cols is None:
                    nc.sync.dma_start(t[:], src[kt * P:(kt + 1) * P, :])
                else:
                    nc.sync.dma_start(t[:], src[kt * P:(kt + 1) * P,
                                                cols * D:(cols + 1) * D])
                return t

            with tc.tile_pool(name="projps", bufs=3, space="PSUM") as pps:
                # transposed-output projections: q, k, k2
                for pname, xn, bias, dst in (("wq", xnA, bq_t, None),
                                             ("wk", xnA, bk_t, kTt),
                                             ("wk2", xnO, bk2_t, k2T)):
                    wt = [load_w(pname, kt) for kt in range(KT)]
                    for mt in range(KT):
                        ps = pps.tile([P, S], f32, name=f"{pname}ps{mt}", tag="pps")
                        for kt in range(KT):
                            st, sp = kt == 0, kt == KT - 1
                            for nh in range(2):
                                sl = slice(nh * 512, (nh + 1) * 512)
                                nc.tensor.matmul(
                                    ps[:, sl],
                                    wt[kt][:, mt * P:(mt + 1) * P],
                                    xn[kt][:, sl], start=st, stop=sp)
                        if dst is not None:
                            nc.vector.tensor_scalar(dst[mt][:], ps[:],
                                                    bias[:, mt:mt + 1], None,
                                                    Ax.add)
                        else:
                            for hh in range(2):
                                h, po = 2 * mt + hh, hh * 64
                                t = qP[h]
                                nc.gpsimd.memset(t[:], 0.0)
                                nc.vector.tensor_scalar(
                                    t[po:po + 64, :], ps[po:po + 64, :],
                                    bias[po:po + 64, mt:mt + 1], None, Ax.add)
                # natural-layout projections with ones column: v, v2
                for pname, xn, bcast, dst in (("wv", xnA, bv_b, vN),
                                              ("wv2", xnO, bv2_b, v2N)):
                    wt = [load_w(pname, kt) for kt in range(KT)]
                    for jt in range(JT):
                        ps = pps.tile([P, D], f32, name=f"{pname}ps{jt}", tag="pps")
                        for kt in range(KT):
                            st, sp = kt == 0, kt == KT - 1
                            for sl in (slice(0, 512), slice(512, 768)):
                                nc.tensor.matmul(
                                    ps[:, sl],
                                    xn[kt][:, jt * P:(jt + 1) * P],
                                    wt[kt][:, sl], start=st, stop=sp)
                        nc.gpsimd.memset(dst[jt][:], 0.0)
                        dst_v = dst[jt][:, 0:NH * 80].rearrange(
                            "p (h c) -> p h c", c=80)
                        nc.vector.tensor_tensor(
                            dst_v[:, :, 0:64],
                            ps.rearrange("p (h c) -> p h c", c=64)[:],
                            bcast.rearrange("p (h c) -> p h c", c=64)[:],
                            Ax.add)
                        nc.gpsimd.memset(dst_v[:, :, 64:65], 1.0)

            if dbg:
                nc.sync.dma_start(dbg["dbgq"][:], qP[0][:])
                nc.sync.dma_start(dbg["dbgk"][:], kTt[0][:])
                nc.sync.dma_start(dbg["dbgv"][:], vN[0][:])

            # ---- attention ----
            wo_t = [load_w("wo", kt) for kt in range(KT)]
            ctx_t = [bbt(f"ctx{t}") for t in range(KT)]
            with tc.tile_pool(name="attnps", bufs=2, space="PSUM") as aps:
                for h in range(NH):
                    td, po = h // 2, (h % 2) * 64
                    tmps = []
                    for typ, (kk, vv) in enumerate(((kTt, vN), (k2T, v2N))):
                        cacc = aps.tile([P, S], f32, name=f"cv{h}_{typ}", tag="cv")
                        for jt in range(JT):
                            sT = aps.tile([P, S], f32, name=f"sT{h}_{typ}_{jt}",
                                          tag="sT")
                            for nh in range(2):
                                sl = slice(nh * 512, (nh + 1) * 512)
                                nc.tensor.matmul(
                                    sT[:, sl],
                                    kk[td][:, jt * P:(jt + 1) * P],
                                    qP[h][:, sl],
                                    start=True, stop=True)
                            pT = bbt(f"pT{h}_{typ}_{jt}")
                            nc.scalar.activation(pT[:], sT[:], Act.Exp)
                            if dbg and h == 0 and typ == 0 and jt == 0:
                                nc.sync.dma_start(dbg["dbgp"][:], pT[:])
                            for nh in range(2):
                                sl = slice(nh * 512, (nh + 1) * 512)
                                nc.tensor.matmul(
                                    cacc[:, sl],
                                    vv[jt][:, h * 80:h * 80 + P],
                                    pT[:, sl],
                                    start=(jt == 0), stop=(jt == JT - 1))
                        recip = fwt(f"rc{h}_{typ}", (1, S))
                        nc.vector.reciprocal(recip[:], cacc[64:65, :])
                        rb = fwt(f"rb{h}_{typ}", (64, S))
                        nc.gpsimd.partition_broadcast(rb[:], recip[:])
                        tmp = bbt(f"tm{h}_{typ}", (64, S))
                        nc.vector.scalar_tensor_tensor(
                            tmp[:], cacc[0:64, :], wsc_b[0:64, typ:typ + 1],
                            rb[:], Ax.mult, Ax.mult)
                        tmps.append(tmp)
                    nc.vector.tensor_add(ctx_t[td][po:po + 64, :],
                                         tmps[0][:], tmps[1][:])
                    if dbg and h == 1:
                        nc.sync.dma_start(dbg["dbgc"][:], ctx_t[0][:])

            # ---- out-projection + residual ----
            x1 = []
            with tc.tile_pool(name="opps", bufs=2, space="PSUM") as ops:
                for mt in range(KT):
                    ps = ops.tile([P, S], f32, name=f"ops{mt}", tag="ops")
                    for kt in range(KT):
                        st, sp = kt == 0, kt == KT - 1
                        for nh in range(2):
                            sl = slice(nh * 512, (nh + 1) * 512)
                            nc.tensor.matmul(ps[:, sl],
                                             wo_t[kt][:, mt * P:(mt + 1) * P],
                                             ctx_t[kt][:, sl], start=st, stop=sp)
                    t = res_pool.tile([P, S], f32, name=f"x1_{mt}", tag="res")
                    nc.vector.scalar_tensor_tensor(t[:], ps[:], bo_t[:, mt:mt + 1],
                                                   aT[mt][:], Ax.add, Ax.add)
                    x1.append(t)

            # ---- LN2 ----
            with tc.tile_pool(name="lnps_b", bufs=1, space="PSUM") as lnps2:
                xn2 = ln_T(x1, None, lnps2, "B")

            # ---- MLP ----
            with tc.tile_pool(name="mlpps", bufs=3, space="PSUM") as mps:
                hbf = []
                for fq in range(4):
                    w1t = [load_w("w1", kt, cols=fq) for kt in range(KT)]
                    for fl in range(KT):
                        ft = fq * KT + fl
                        ps = mps.tile([P, S], f32, name=f"h_ps{ft}", tag="mps")
                        for kt in range(KT):
                            st, sp = kt == 0, kt == KT - 1
                            for nh in range(2):
                                sl = slice(nh * 512, (nh + 1) * 512)
                                nc.tensor.matmul(ps[:, sl],
                                                 w1t[kt][:, fl * P:(fl + 1) * P],
                                                 xn2[kt][:, sl], start=st, stop=sp)
                        hb = bbt(f"hbf{ft}")
                        nc.scalar.activation(hb[:], ps[:], Act.Gelu_apprx_tanh,
                                             bias=b1_t[:, ft:ft + 1])
                        hbf.append(hb)
                for half in range(2):
                    psl = []
                    for ml in range(3):
                        ps = mps.tile([P, S], f32, name=f"o_ps{half}_{ml}",
                                      tag="mps")
                        psl.append(ps)
                    for kt in range(FT):
                        w2t = load_w("w2", kt)
                        for ml in range(3):
                            mt = half * 3 + ml
                            st, sp = kt == 0, kt == FT - 1
                            for nh in range(2):
                                sl = slice(nh * 512, (nh + 1) * 512)
                                nc.tensor.matmul(psl[ml][:, sl],
                                                 w2t[:, mt * P:(mt + 1) * P],
                                                 hbf[kt][:, sl], start=st, stop=sp)
                    for ml in range(3):
                        mt = half * 3 + ml
                        ot = fwt(f"out{mt}")
                        nc.vector.scalar_tensor_tensor(ot[:], psl[ml][:],
                                                       b2_t[:, mt:mt + 1],
                                                       x1[mt][:], Ax.add, Ax.add)
                        nc.sync.dma_start(out_d[mt * P:(mt + 1) * P, :], ot[:])

    nc.compile()
    return nc


def _get_program():
    global _PROGRAM
    if _PROGRAM is None:
        _PROGRAM = _build_program()
    return _PROGRAM


def _fold_core(inp, b, s):
    """Host-side shard + weight folding for core (batch b, stream s)."""
    if s == 0:
        a, o = inp['x'][b], inp['y'][b]
        g1s, b1s, g1o, b1o = inp['ln1x_g'], inp['ln1x_b'], inp['ln1y_g'], inp['ln1y_b']
        Wq, bq, Wk, bk, Wv, bv = inp['Wq'], inp['bq'], inp['Wk'], inp['bk'], inp['Wv'], inp['bv']
        Wk2, bk2, Wv2, bv2 = inp['Wkd'], inp['bkd'], inp['Wvd'], inp['bvd']
        Wo, bo = inp['Wo'], inp['bo']
        ws, wc = inp['w11'][0], inp['w12'][0]
        g2, b2g = inp['ln2x_g'], inp['ln2x_b']
        W1, b1, W2, b2 = inp['W1'], inp['b1'], inp['W2'], inp['b2']
    else:
        a, o = inp['y'][b], inp['x'][b]
        g1s, b1s, g1o, b1o = inp['ln1y_g'], inp['ln1y_b'], inp['ln1x_g'], inp['ln1x_b']
        Wq, bq, Wk, bk, Wv, bv = inp['Wqd'], inp['bqd'], inp['Wkd'], inp['bkd'], inp['Wvd'], inp['bvd']
        Wk2, bk2, Wv2, bv2 = inp['Wk'], inp['bk'], inp['Wv'], inp['bv']
        Wo, bo = inp['Wod'], inp['bod']
        ws, wc = inp['w21'][0], inp['w22'][0]
        g2, b2g = inp['ln2y_g'], inp['ln2y_b']
        W1, b1, W2, b2 = inp['W1d'], inp['b1d'], inp['W2d'], inp['b2d']

    m = {
        'aT': np.ascontiguousarray(a.T, np.float32),
        'aTb': np.ascontiguousarray(a.T).astype(BF),
        'oT': np.ascontiguousarray(o.T).astype(BF),
        'wq': np.ascontiguousarray(g1s[:, None] * Wq * SCALE).astype(BF),
        'wk': np.ascontiguousarray(g1s[:, None] * Wk).astype(BF),
        'wv': np.ascontiguousarray(g1s[:, None] * Wv).astype(BF),
        'wk2': np.ascontiguousarray(g1o[:, None] * Wk2).astype(BF),
        'wv2': np.ascontiguousarray(g1o[:, None] * Wv2).astype(BF),
        'wo': np.ascontiguousarray(Wo).astype(BF),
        'w1': np.ascontiguousarray(g2[:, None] * W1).astype(BF),
        'w2': np.ascontiguousarray(W2).astype(BF),
        'bq': (SCALE * (bq + b1s @ Wq)).astype(np.float32).reshape(D, 1),
        'bk': (bk + b1s @ Wk).astype(np.float32).reshape(D, 1),
        'bk2': (bk2 + b1o @ Wk2).astype(np.float32).reshape(D, 1),
        'bo': ((ws + wc) * bo).astype(np.float32).reshape(D, 1),
        'b2': np.asarray(b2, np.float32).reshape(D, 1),
        'b1': (b1 + b2g @ W1).astype(np.float32).reshape(F, 1),
        'bvrow': (bv + b1s @ Wv).astype(np.float32).reshape(1, D).astype(BF),
        'bv2row': (bv2 + b1o @ Wv2).astype(np.float32).reshape(1, D).astype(BF),
        'wsc': np.array([[ws, wc]], np.float32),
    }
    return m


LAST_RESULTS = None


def kernel(**inputs):
    from concourse.bass_utils import run_bass_kernel_spmd
    global LAST_RESULTS

    inp = {k: np.asarray(v, np.float32) for k, v in inputs.items()}
    B = inp['x'].shape[0]

    nc = _get_program()
    in_maps = [_fold_core(inp, core // 2, core % 2) for core in range(2 * B)]
    res = run_bass_kernel_spmd(
        nc, in_maps, core_ids=list(range(2 * B)),
        trace=bool(os.environ.get("KERNEL_TRACE")))
    LAST_RESULTS = res

    x_out = np.empty((B, S, D), np.float32)
    y_out = np.empty((B, S, D), np.float32)
    for b in range(B):
        x_out[b] = res.results[2 * b]["outT"].T
        y_out[b] = res.results[2 * b + 1]["outT"].T
    return (x_out, y_out)
